# revision 6
# baseline (speedup 1.0000x reference)
"""Trainium2 Bass kernel for nn_Dereverber_v3 (ragged MLP + masked mean-pool + MLP).

Strategy (pure data parallelism over 8 NeuronCores):
- Shard samples: core c gets samples [512c, 512(c+1)).
- Host: zero out invalid (s, m >= numInputs[s]) slots of all_x, transpose to
  feature-major [512pad, 16384] per core, pad feature dims to multiples of 128.
- Device per core: h1 = relu(W1 xT + b1), h2 = relu(W2 h1 + b2) with activations
  kept transposed [feat, rows]; ragged mean-pool computed as
  (sum over all 32 slots - (32 - n_s) * c2) / n_s, where c2 = MLP(0) is computed
  on device from a zero column (exact cancellation of padded slots);
  then h3 = relu(W3 pooled + b3), y = relu(W4 h3 + b4) with the last matmul
  flipped so output lands sample-major for direct DMA out.
- All matmuls run in float32r (full PE rate, ~1e-4 rel err).
"""

import numpy as np
import concourse.bass as bass  # noqa: F401  (engine namespaces live on the nc object)
import concourse.mybir as mybir
import concourse.tile as tile
from concourse import bacc
from concourse.bass_utils import run_bass_kernel_spmd

S, M, D, H = 4096, 32, 420, 840
N_CORES = 8
SC = S // N_CORES            # samples per core (512)
R = SC * M                   # slot-rows per core (16384)
TILE_N = 512                 # rows per tile
N_TILES = R // TILE_N        # 32
SAMP_PER_TILE = TILE_N // M  # 16
DP = 512                     # padded feature dim (d, e, f): 4 chunks of 128
HP = 896                     # padded hidden dim (h): 7 chunks of 128
NKD = DP // 128              # 4
NKH = HP // 128              # 7

F32 = mybir.dt.float32
F32R = mybir.dt.float32r

LAST_EXEC_NS = None
_PROGRAM_CACHE = {}


def _build_program():
    nc = bacc.Bacc("TRN2", target_bir_lowering=False, debug=False)

    xT = nc.dram_tensor("xT", [DP, R], F32R, kind="ExternalInput").ap()
    w1t = nc.dram_tensor("w1t", [DP, DP], F32R, kind="ExternalInput").ap()
    w2t = nc.dram_tensor("w2t", [DP, DP], F32R, kind="ExternalInput").ap()
    w3t = nc.dram_tensor("w3t", [DP, HP], F32R, kind="ExternalInput").ap()
    w4t = nc.dram_tensor("w4t", [HP, DP], F32R, kind="ExternalInput").ap()
    b1c = nc.dram_tensor("b1c", [128, NKD], F32, kind="ExternalInput").ap()
    b2c = nc.dram_tensor("b2c", [128, NKD], F32, kind="ExternalInput").ap()
    b3c = nc.dram_tensor("b3c", [128, NKH], F32, kind="ExternalInput").ap()
    b4rep = nc.dram_tensor("b4rep", [128, D], F32, kind="ExternalInput").ap()
    invrep = nc.dram_tensor("invrep", [128, SC], F32, kind="ExternalInput").ap()
    betarep = nc.dram_tensor("betarep", [128, SC], F32, kind="ExternalInput").ap()
    zc = nc.dram_tensor("zc", [128, NKD * 256], F32R, kind="ExternalInput").ap()
    y = nc.dram_tensor("y", [SC, D], F32, kind="ExternalOutput").ap()

    relu = mybir.ActivationFunctionType.Relu

    with tile.TileContext(nc) as tc:
        with (
            tc.tile_pool(name="const", bufs=1) as const,
            tc.tile_pool(name="acc", bufs=1) as acc,
            tc.tile_pool(name="xin", bufs=3) as xin,
            tc.tile_pool(name="work", bufs=2) as work,
            tc.tile_pool(name="psum", bufs=6, space="PSUM") as psum,
        ):
            # ---- constants to SBUF ----
            w1t_sb = const.tile([128, NKD, DP], F32R)
            w2t_sb = const.tile([128, NKD, DP], F32R)
            w3t_sb = const.tile([128, NKD, HP], F32R)
            w4t_sb = const.tile([128, NKH, DP], F32R)
            nc.sync.dma_start(w1t_sb[:], w1t.rearrange("(c p) e -> p c e", p=128))
            nc.sync.dma_start(w2t_sb[:], w2t.rearrange("(c p) e -> p c e", p=128))
            nc.sync.dma_start(w3t_sb[:], w3t.rearrange("(c p) e -> p c e", p=128))
            nc.sync.dma_start(w4t_sb[:], w4t.rearrange("(c p) e -> p c e", p=128))
            b1_sb = const.tile([128, NKD], F32)
            b2_sb = const.tile([128, NKD], F32)
            b3_sb = const.tile([128, NKH], F32)
            b4_sb = const.tile([128, D], F32)
            inv_sb = const.tile([128, SC], F32)
            beta_sb = const.tile([128, SC], F32)
            nc.sync.dma_start(b1_sb[:], b1c)
            nc.sync.dma_start(b2_sb[:], b2c)
            nc.sync.dma_start(b3_sb[:], b3c)
            nc.sync.dma_start(b4_sb[:], b4rep)
            nc.sync.dma_start(inv_sb[:], invrep)
            nc.sync.dma_start(beta_sb[:], betarep)

            # ---- c2 = layer2(layer1(0)) on device (exact pad-slot cancellation) ----
            zcol = const.tile([128, NKD, 256], F32R)
            nc.sync.dma_start(zcol[:], zc.rearrange("p (c n) -> p c n", c=NKD))
            h1c = const.tile([128, NKD, 256], F32R)
            c2_sb = const.tile([128, NKD], F32)
            for ec in range(NKD):
                ps = psum.tile([128, 512], F32, tag="ps")
                for kc in range(NKD):
                    nc.tensor.matmul(
                        ps[:, :256],
                        w1t_sb[:, kc, ec * 128:(ec + 1) * 128],
                        zcol[:, kc, :],
                        start=(kc == 0), stop=(kc == NKD - 1),
                    )
                nc.scalar.activation(h1c[:, ec, :], ps[:, :256], relu,
                                     bias=b1_sb[:, ec:ec + 1])
            for fc in range(NKD):
                ps = psum.tile([128, 512], F32, tag="ps")
                for ec in range(NKD):
                    nc.tensor.matmul(
                        ps[:, :256],
                        w2t_sb[:, ec, fc * 128:(fc + 1) * 128],
                        h1c[:, ec, :],
                        start=(ec == 0), stop=(ec == NKD - 1),
                    )
                nc.scalar.activation(c2_sb[:, fc:fc + 1], ps[:, :1], relu,
                                     bias=b2_sb[:, fc:fc + 1])

            # ---- main loop: layers 1+2 + slot-sum per 512-row tile ----
            sumT = acc.tile([128, NKD, SC], F32)
            for t in range(N_TILES):
                n0 = t * TILE_N
                x_t = xin.tile([128, NKD, TILE_N], F32R, tag="x")
                nc.sync.dma_start(
                    x_t[:], xT[:, n0:n0 + TILE_N].rearrange("(c p) n -> p c n", p=128)
                )
                h1_t = work.tile([128, NKD, TILE_N], F32R, tag="h1")
                for ec in range(NKD):
                    ps = psum.tile([128, 512], F32, tag="ps")
                    for kc in range(NKD):
                        nc.tensor.matmul(
                            ps[:],
                            w1t_sb[:, kc, ec * 128:(ec + 1) * 128],
                            x_t[:, kc, :],
                            start=(kc == 0), stop=(kc == NKD - 1),
                        )
                    nc.scalar.activation(h1_t[:, ec, :], ps[:], relu,
                                         bias=b1_sb[:, ec:ec + 1])
                for fc in range(NKD):
                    ps = psum.tile([128, 512], F32, tag="ps")
                    for ec in range(NKD):
                        nc.tensor.matmul(
                            ps[:],
                            w2t_sb[:, ec, fc * 128:(fc + 1) * 128],
                            h1_t[:, ec, :],
                            start=(ec == 0), stop=(ec == NKD - 1),
                        )
                    h2_t = work.tile([128, TILE_N], F32, tag="h2")
                    nc.scalar.activation(h2_t[:], ps[:], relu,
                                         bias=b2_sb[:, fc:fc + 1])
                    nc.vector.reduce_sum(
                        sumT[:, fc, t * SAMP_PER_TILE:(t + 1) * SAMP_PER_TILE],
                        h2_t[:].rearrange("p (s m) -> p s m", m=M),
                        axis=mybir.AxisListType.X,
                    )

            # ---- ragged correction: pooled = sum * inv_n - beta * c2 ----
            pooledT = acc.tile([128, NKD, SC], F32R)
            for fc in range(NKD):
                t1 = work.tile([128, SC], F32, tag="t1")
                t2 = work.tile([128, SC], F32, tag="t2")
                nc.vector.tensor_mul(
                    t1[:], beta_sb[:],
                    c2_sb[:, fc:fc + 1].to_broadcast((128, SC)),
                )
                nc.vector.tensor_mul(t2[:], sumT[:, fc, :], inv_sb[:])
                nc.vector.tensor_sub(pooledT[:, fc, :], t2[:], t1[:])

            # ---- layer 3: h3T = relu(W3 pooled + b3), [HP, SC] ----
            h3T = acc.tile([128, NKH, SC], F32R)
            for hc in range(NKH):
                ps = psum.tile([128, 512], F32, tag="ps")
                for kc in range(NKD):
                    nc.tensor.matmul(
                        ps[:],
                        w3t_sb[:, kc, hc * 128:(hc + 1) * 128],
                        pooledT[:, kc, :],
                        start=(kc == 0), stop=(kc == NKD - 1),
                    )
                nc.scalar.activation(h3T[:, hc, :], ps[:], relu,
                                     bias=b3_sb[:, hc:hc + 1])

            # ---- layer 4 (flipped): y[s, :] = relu(h3T.T @ W4T + b4) ----
            for sc4 in range(SC // 128):
                ps = psum.tile([128, 512], F32, tag="ps")
                for hc in range(NKH):
                    nc.tensor.matmul(
                        ps[:, :D],
                        h3T[:, hc, sc4 * 128:(sc4 + 1) * 128],
                        w4t_sb[:, hc, :D],
                        start=(hc == 0), stop=(hc == NKH - 1),
                    )
                y_t = work.tile([128, D], F32, tag="yt")
                nc.vector.tensor_add(y_t[:], ps[:, :D], b4_sb[:])
                nc.vector.tensor_scalar_max(y_t[:], y_t[:], 0.0)
                nc.sync.dma_start(y[sc4 * 128:(sc4 + 1) * 128, :], y_t[:])

    nc.compile()
    return nc


def kernel(**inputs):
    global LAST_EXEC_NS
    all_x = np.asarray(inputs["all_x"], dtype=np.float32)
    numInputs = np.asarray(inputs["numInputs"]).astype(np.int64)
    W1 = np.asarray(inputs["W1"], dtype=np.float32)
    b1 = np.asarray(inputs["b1"], dtype=np.float32)
    W2 = np.asarray(inputs["W2"], dtype=np.float32)
    b2 = np.asarray(inputs["b2"], dtype=np.float32)
    W3 = np.asarray(inputs["W3"], dtype=np.float32)
    b3 = np.asarray(inputs["b3"], dtype=np.float32)
    W4 = np.asarray(inputs["W4"], dtype=np.float32)
    b4 = np.asarray(inputs["b4"], dtype=np.float32)

    # zero invalid slots so the device-side slot-sum + c2 correction is exact
    slot_valid = (np.arange(M)[None, :] < numInputs[:, None])  # [S, M]
    xz = np.where(slot_valid[:, :, None], all_x, np.float32(0.0))

    def padded_T(w, rows_to, cols_to=None):
        # w [out, in] -> transposed+padded [rows_to(in), cols_to(out)]
        out_dim, in_dim = w.shape
        cols_to = cols_to or out_dim
        a = np.zeros((rows_to, cols_to), np.float32)
        a[:in_dim, :out_dim] = w.T
        return a

    w1t = padded_T(W1, DP, DP)
    w2t = padded_T(W2, DP, DP)
    w3t = padded_T(W3, DP, HP)
    w4t = padded_T(W4, HP, DP)

    def chunked_bias(b, pad_to, nchunks):
        a = np.zeros(pad_to, np.float32)
        a[:b.shape[0]] = b
        return np.ascontiguousarray(a.reshape(nchunks, 128).T)  # [128, nchunks]

    b1c = chunked_bias(b1, DP, NKD)
    b2c = chunked_bias(b2, DP, NKD)
    b3c = chunked_bias(b3, HP, NKH)
    b4rep = np.ascontiguousarray(np.broadcast_to(b4[None, :], (128, D)))

    nc = _PROGRAM_CACHE.get("nc")
    if nc is None:
        nc = _build_program()
        _PROGRAM_CACHE["nc"] = nc

    in_maps = []
    for c in range(N_CORES):
        s0 = c * SC
        xc = xz[s0:s0 + SC].reshape(R, D)  # [16384, 420]
        xT = np.zeros((DP, R), np.float32)
        xT[:D, :] = xc.T
        n_c = numInputs[s0:s0 + SC].astype(np.float32)
        inv = (1.0 / n_c).astype(np.float32)
        beta = ((M - n_c) / n_c).astype(np.float32)
        in_maps.append({
            "xT": xT,
            "w1t": w1t, "w2t": w2t, "w3t": w3t, "w4t": w4t,
            "b1c": b1c, "b2c": b2c, "b3c": b3c, "b4rep": b4rep,
            "zc": np.zeros((128, NKD * 256), np.float32),
            "invrep": np.ascontiguousarray(np.broadcast_to(inv[None, :], (128, SC))),
            "betarep": np.ascontiguousarray(np.broadcast_to(beta[None, :], (128, SC))),
        })

    res = run_bass_kernel_spmd(nc, in_maps, core_ids=list(range(N_CORES)))
    LAST_EXEC_NS = res.exec_time_ns

    out = np.concatenate([res.results[c]["y"] for c in range(N_CORES)], axis=0)
    return out[:, None, :].astype(np.float32)


# revision 7
# speedup vs baseline: 1.6231x; 1.6231x over previous
"""Trainium2 Bass kernel for nn_Dereverber_v3 (ragged MLP + masked mean-pool + MLP).

Strategy (pure data parallelism over 8 NeuronCores, ragged-packed):
- Sort samples by numInputs descending, deal round-robin to the 8 cores so all
  cores share one compile-time row geometry (per-core padding <= 32 rows).
- Pack valid (sample, slot) rows slot-major into "planes": plane j holds
  column s for every sample rank s with numInputs > j; plane widths cnt_j are
  a shared prefix staircase. Invalid/pad rows are zeroed.
- Device per core: h1 = relu(W1 xT + b1), h2 = relu(W2 h1 + b2) over packed
  rows only (~52% of dense), activations transposed [feat, rows].
- Mean-pool: acc[f, s] = sum over planes of h2 columns (DVE adds over the
  plane staircase); pad columns contribute c2 = MLP2(0), computed on device
  from a zero column, cancelled exactly: pooled = (acc - (B_s - n_s) c2)/n_s.
- h3 = relu(W3 pooled + b3); y = relu(W4 h3 + b4) with the last matmul flipped
  so output lands sample-major; host undoes the sort/deal permutation.
- All matmuls in float32r (full PE rate, ~1e-4 rel err).
"""

import numpy as np
import concourse.bass as bass  # noqa: F401
import concourse.mybir as mybir
import concourse.tile as tile
from concourse import bacc
from concourse.bass_utils import run_bass_kernel_spmd

S, M, D, H = 4096, 32, 420, 840
N_CORES = 8
SC = S // N_CORES            # samples per core (512)
TILE_N = 512                 # rows per tile
DP = 512                     # padded feature dim (d, e, f): 4 chunks of 128
HP = 896                     # padded hidden dim (h): 7 chunks of 128
NKD = DP // 128              # 4
NKH = HP // 128              # 7

F32 = mybir.dt.float32
F32R = mybir.dt.float32r

LAST_EXEC_NS = None
_PROGRAM_CACHE = {}


def _build_program(cnt):
    """cnt: tuple of plane widths (len M, non-increasing). Shared across cores."""
    cnt = [c for c in cnt if c > 0]
    offs = np.concatenate([[0], np.cumsum(cnt)])  # plane start offsets
    rows = int(offs[-1])
    r_pad = -(-rows // TILE_N) * TILE_N
    n_tiles = r_pad // TILE_N

    # tile t -> list of (fc-independent) adds: (j, col0, col1, tilecol0)
    tile_adds = [[] for _ in range(n_tiles)]
    for j, w in enumerate(cnt):
        o = int(offs[j])
        t0, t1 = o // TILE_N, (o + w - 1) // TILE_N
        for t in range(t0, t1 + 1):
            s0 = max(0, t * TILE_N - o)
            s1 = min(w, (t + 1) * TILE_N - o)
            tile_adds[t].append((s0, s1, o + s0 - t * TILE_N))

    nc = bacc.Bacc("TRN2", target_bir_lowering=False, debug=False)

    xT = nc.dram_tensor("xT", [DP, r_pad], F32R, kind="ExternalInput").ap()
    w1t = nc.dram_tensor("w1t", [DP, DP], F32R, kind="ExternalInput").ap()
    w2t = nc.dram_tensor("w2t", [DP, DP], F32R, kind="ExternalInput").ap()
    w3t = nc.dram_tensor("w3t", [DP, HP], F32R, kind="ExternalInput").ap()
    w4t = nc.dram_tensor("w4t", [HP, DP], F32R, kind="ExternalInput").ap()
    b1c = nc.dram_tensor("b1c", [128, NKD], F32, kind="ExternalInput").ap()
    b2c = nc.dram_tensor("b2c", [128, NKD], F32, kind="ExternalInput").ap()
    b3c = nc.dram_tensor("b3c", [128, NKH], F32, kind="ExternalInput").ap()
    b4rep = nc.dram_tensor("b4rep", [128, D], F32, kind="ExternalInput").ap()
    invrep = nc.dram_tensor("invrep", [128, SC], F32, kind="ExternalInput").ap()
    betarep = nc.dram_tensor("betarep", [128, SC], F32, kind="ExternalInput").ap()
    zc = nc.dram_tensor("zc", [128, NKD * 256], F32R, kind="ExternalInput").ap()
    y = nc.dram_tensor("y", [SC, D], F32, kind="ExternalOutput").ap()

    relu = mybir.ActivationFunctionType.Relu

    with tile.TileContext(nc) as tc:
        with (
            tc.tile_pool(name="const", bufs=1) as const,
            tc.tile_pool(name="acc", bufs=1) as accp,
            tc.tile_pool(name="xin", bufs=3) as xin,
            tc.tile_pool(name="work", bufs=2) as work,
            tc.tile_pool(name="h2p", bufs=3) as h2p,
            tc.tile_pool(name="psum", bufs=6, space="PSUM") as psum,
        ):
            # ---- constants to SBUF ----
            w1t_sb = const.tile([128, NKD, DP], F32R)
            w2t_sb = const.tile([128, NKD, DP], F32R)
            w3t_sb = const.tile([128, NKD, HP], F32R)
            w4t_sb = const.tile([128, NKH, DP], F32R)
            nc.sync.dma_start(w1t_sb[:], w1t.rearrange("(c p) e -> p c e", p=128))
            nc.sync.dma_start(w2t_sb[:], w2t.rearrange("(c p) e -> p c e", p=128))
            nc.sync.dma_start(w3t_sb[:], w3t.rearrange("(c p) e -> p c e", p=128))
            nc.sync.dma_start(w4t_sb[:], w4t.rearrange("(c p) e -> p c e", p=128))
            b1_sb = const.tile([128, NKD], F32)
            b2_sb = const.tile([128, NKD], F32)
            b3_sb = const.tile([128, NKH], F32)
            b4_sb = const.tile([128, D], F32)
            inv_sb = const.tile([128, SC], F32)
            beta_sb = const.tile([128, SC], F32)
            nc.sync.dma_start(b1_sb[:], b1c)
            nc.sync.dma_start(b2_sb[:], b2c)
            nc.sync.dma_start(b3_sb[:], b3c)
            nc.sync.dma_start(b4_sb[:], b4rep)
            nc.sync.dma_start(inv_sb[:], invrep)
            nc.sync.dma_start(beta_sb[:], betarep)

            # ---- c2 = layer2(layer1(0)) on device (exact pad cancellation) ----
            zcol = const.tile([128, NKD, 256], F32R)
            nc.sync.dma_start(zcol[:], zc.rearrange("p (c n) -> p c n", c=NKD))
            h1c = const.tile([128, NKD, 256], F32R)
            c2_sb = const.tile([128, NKD], F32)
            for ec in range(NKD):
                ps = psum.tile([128, 512], F32, tag="ps")
                for kc in range(NKD):
                    nc.tensor.matmul(
                        ps[:, :256],
                        w1t_sb[:, kc, ec * 128:(ec + 1) * 128],
                        zcol[:, kc, :],
                        start=(kc == 0), stop=(kc == NKD - 1),
                    )
                nc.scalar.activation(h1c[:, ec, :], ps[:, :256], relu,
                                     bias=b1_sb[:, ec:ec + 1])
            for fc in range(NKD):
                ps = psum.tile([128, 512], F32, tag="ps")
                for ec in range(NKD):
                    nc.tensor.matmul(
                        ps[:, :256],
                        w2t_sb[:, ec, fc * 128:(fc + 1) * 128],
                        h1c[:, ec, :],
                        start=(ec == 0), stop=(ec == NKD - 1),
                    )
                nc.scalar.activation(c2_sb[:, fc:fc + 1], ps[:, :1], relu,
                                     bias=b2_sb[:, fc:fc + 1])

            # ---- pooled-sum accumulator ----
            acc = accp.tile([128, NKD, SC], F32)
            nc.any.memset(acc[:], 0.0)

            # ---- main loop over packed row tiles ----
            for t in range(n_tiles):
                n0 = t * TILE_N
                x_t = xin.tile([128, NKD, TILE_N], F32R, tag="x")
                nc.sync.dma_start(
                    x_t[:], xT[:, n0:n0 + TILE_N].rearrange("(c p) n -> p c n", p=128)
                )
                h1_t = work.tile([128, NKD, TILE_N], F32R, tag="h1")
                for ec in range(NKD):
                    ps = psum.tile([128, 512], F32, tag="ps")
                    for kc in range(NKD):
                        nc.tensor.matmul(
                            ps[:],
                            w1t_sb[:, kc, ec * 128:(ec + 1) * 128],
                            x_t[:, kc, :],
                            start=(kc == 0), stop=(kc == NKD - 1),
                        )
                    nc.scalar.activation(h1_t[:, ec, :], ps[:], relu,
                                         bias=b1_sb[:, ec:ec + 1])
                for fc in range(NKD):
                    ps = psum.tile([128, 512], F32, tag="ps")
                    for ec in range(NKD):
                        nc.tensor.matmul(
                            ps[:],
                            w2t_sb[:, ec, fc * 128:(fc + 1) * 128],
                            h1_t[:, ec, :],
                            start=(ec == 0), stop=(ec == NKD - 1),
                        )
                    h2_t = h2p.tile([128, TILE_N], F32, tag="h2")
                    nc.scalar.activation(h2_t[:], ps[:], relu,
                                         bias=b2_sb[:, fc:fc + 1])
                    for (s0, s1, tc0) in tile_adds[t]:
                        w = s1 - s0
                        nc.vector.tensor_add(
                            acc[:, fc, s0:s1],
                            acc[:, fc, s0:s1],
                            h2_t[:, tc0:tc0 + w],
                        )

            # ---- ragged correction: pooled = acc * inv_n - beta * c2 ----
            pooledT = accp.tile([128, NKD, SC], F32R)
            for fc in range(NKD):
                t1 = work.tile([128, SC], F32, tag="t1")
                t2 = work.tile([128, SC], F32, tag="t2")
                nc.vector.tensor_mul(
                    t1[:], beta_sb[:],
                    c2_sb[:, fc:fc + 1].to_broadcast((128, SC)),
                )
                nc.vector.tensor_mul(t2[:], acc[:, fc, :], inv_sb[:])
                nc.vector.tensor_sub(pooledT[:, fc, :], t2[:], t1[:])

            # ---- layer 3: h3T = relu(W3 pooled + b3), [HP, SC] ----
            h3T = accp.tile([128, NKH, SC], F32R)
            for hc in range(NKH):
                ps = psum.tile([128, 512], F32, tag="ps")
                for kc in range(NKD):
                    nc.tensor.matmul(
                        ps[:],
                        w3t_sb[:, kc, hc * 128:(hc + 1) * 128],
                        pooledT[:, kc, :],
                        start=(kc == 0), stop=(kc == NKD - 1),
                    )
                nc.scalar.activation(h3T[:, hc, :], ps[:], relu,
                                     bias=b3_sb[:, hc:hc + 1])

            # ---- layer 4 (flipped): y[s, :] = relu(h3T.T @ W4T + b4) ----
            for sc4 in range(SC // 128):
                ps = psum.tile([128, 512], F32, tag="ps")
                for hc in range(NKH):
                    nc.tensor.matmul(
                        ps[:, :D],
                        h3T[:, hc, sc4 * 128:(sc4 + 1) * 128],
                        w4t_sb[:, hc, :D],
                        start=(hc == 0), stop=(hc == NKH - 1),
                    )
                y_t = work.tile([128, D], F32, tag="yt")
                nc.vector.tensor_add(y_t[:], ps[:, :D], b4_sb[:])
                nc.vector.tensor_scalar_max(y_t[:], y_t[:], 0.0)
                nc.sync.dma_start(y[sc4 * 128:(sc4 + 1) * 128, :], y_t[:])

    nc.compile()
    return nc, rows, r_pad


def kernel(**inputs):
    global LAST_EXEC_NS
    all_x = np.asarray(inputs["all_x"], dtype=np.float32)
    numInputs = np.asarray(inputs["numInputs"]).astype(np.int64)
    W1 = np.asarray(inputs["W1"], dtype=np.float32)
    b1 = np.asarray(inputs["b1"], dtype=np.float32)
    W2 = np.asarray(inputs["W2"], dtype=np.float32)
    b2 = np.asarray(inputs["b2"], dtype=np.float32)
    W3 = np.asarray(inputs["W3"], dtype=np.float32)
    b3 = np.asarray(inputs["b3"], dtype=np.float32)
    W4 = np.asarray(inputs["W4"], dtype=np.float32)
    b4 = np.asarray(inputs["b4"], dtype=np.float32)

    # ---- shared ragged geometry: sort by n desc, deal round-robin ----
    perm = np.argsort(-numInputs, kind="stable")      # global rank -> sample
    G = np.array([(numInputs > j).sum() for j in range(M)])  # plane sizes
    cnt = np.ceil(G / N_CORES).astype(np.int64)       # shared per-core widths
    cnt_key = tuple(int(c) for c in cnt)
    B = np.array([(cnt > s).sum() for s in range(SC)], np.int64)  # slots/column

    cached = _PROGRAM_CACHE.get(cnt_key)
    if cached is None:
        cached = _build_program(cnt_key)
        _PROGRAM_CACHE[cnt_key] = cached
    nc, rows, r_pad = cached

    def padded_T(w, rows_to, cols_to=None):
        out_dim, in_dim = w.shape
        cols_to = cols_to or out_dim
        a = np.zeros((rows_to, cols_to), np.float32)
        a[:in_dim, :out_dim] = w.T
        return a

    w1t = padded_T(W1, DP, DP)
    w2t = padded_T(W2, DP, DP)
    w3t = padded_T(W3, DP, HP)
    w4t = padded_T(W4, HP, DP)

    def chunked_bias(b, pad_to, nchunks):
        a = np.zeros(pad_to, np.float32)
        a[:b.shape[0]] = b
        return np.ascontiguousarray(a.reshape(nchunks, 128).T)

    b1c = chunked_bias(b1, DP, NKD)
    b2c = chunked_bias(b2, DP, NKD)
    b3c = chunked_bias(b3, HP, NKH)
    b4rep = np.ascontiguousarray(np.broadcast_to(b4[None, :], (128, D)))

    x_flat = all_x.reshape(S * M, D)
    x_flat = np.vstack([x_flat, np.zeros((1, D), np.float32)])  # row -1 = zeros
    cnz = [j for j in range(M) if cnt[j] > 0]

    in_maps = []
    for c in range(N_CORES):
        # build packed row index list for this core
        idx_parts = []
        for j in cnz:
            w = int(cnt[j])
            scols = np.arange(w)
            ranks = N_CORES * scols + c
            valid = ranks < G[j]
            samp = np.where(valid, perm[np.minimum(ranks, S - 1)], -1)
            fi = np.where(valid, samp * M + j, S * M)  # S*M -> zero row
            idx_parts.append(fi)
        idx = np.concatenate(idx_parts)
        xp = x_flat[idx]                               # [rows, D]
        xT = np.zeros((DP, r_pad), np.float32)
        xT[:D, :rows] = xp.T

        ranks_c = N_CORES * np.arange(SC) + c
        n_col = numInputs[perm[ranks_c]].astype(np.float32)
        inv = (1.0 / n_col).astype(np.float32)
        beta = ((B - n_col) / n_col).astype(np.float32)
        in_maps.append({
            "xT": xT,
            "w1t": w1t, "w2t": w2t, "w3t": w3t, "w4t": w4t,
            "b1c": b1c, "b2c": b2c, "b3c": b3c, "b4rep": b4rep,
            "zc": np.zeros((128, NKD * 256), np.float32),
            "invrep": np.ascontiguousarray(np.broadcast_to(inv[None, :], (128, SC))),
            "betarep": np.ascontiguousarray(np.broadcast_to(beta[None, :], (128, SC))),
        })

    res = run_bass_kernel_spmd(nc, in_maps, core_ids=list(range(N_CORES)))
    LAST_EXEC_NS = res.exec_time_ns

    out = np.empty((S, D), np.float32)
    for c in range(N_CORES):
        ranks_c = N_CORES * np.arange(SC) + c
        out[perm[ranks_c]] = res.results[c]["y"]
    return out[:, None, :].astype(np.float32)


# revision 8
# speedup vs baseline: 1.6264x; 1.0021x over previous
"""Trainium2 Bass kernel for nn_Dereverber_v3 (ragged MLP + masked mean-pool + MLP).

Strategy (pure data parallelism over 8 NeuronCores, ragged-packed):
- Sort samples by numInputs descending, deal round-robin to the 8 cores so all
  cores share one compile-time row geometry (per-core padding <= 32 rows).
- Pack valid (sample, slot) rows slot-major into "planes": plane j holds
  column s for every sample rank s with numInputs > j; plane widths cnt_j are
  a shared prefix staircase. Invalid/pad rows are zeroed.
- Device per core: h1 = relu(W1 xT + b1), h2 = relu(W2 h1 + b2) over packed
  rows only (~52% of dense), activations transposed [feat, rows].
- Mean-pool: acc[f, s] = sum over planes of h2 columns (DVE adds over the
  plane staircase); pad columns contribute c2 = MLP2(0), computed on device
  from a zero column, cancelled exactly: pooled = (acc - (B_s - n_s) c2)/n_s.
- h3 = relu(W3 pooled + b3); y = relu(W4 h3 + b4) with the last matmul flipped
  so output lands sample-major; host undoes the sort/deal permutation.
- All matmuls in float32r (full PE rate, ~1e-4 rel err).
"""

import numpy as np
import concourse.bass as bass  # noqa: F401
import concourse.mybir as mybir
import concourse.tile as tile
from concourse import bacc
from concourse.bass_utils import run_bass_kernel_spmd

S, M, D, H = 4096, 32, 420, 840
N_CORES = 8
SC = S // N_CORES            # samples per core (512)
TILE_N = 512                 # rows per tile
DP = 512                     # padded feature dim (d, e, f): 4 chunks of 128
HP = 896                     # padded hidden dim (h): 7 chunks of 128
NKD = DP // 128              # 4
NKH = HP // 128              # 7

F32 = mybir.dt.float32
F32R = mybir.dt.float32r

LAST_EXEC_NS = None
_PROGRAM_CACHE = {}


def _build_program(cnt):
    """cnt: tuple of plane widths (len M, non-increasing). Shared across cores."""
    cnt = [c for c in cnt if c > 0]
    offs = np.concatenate([[0], np.cumsum(cnt)])  # plane start offsets
    rows = int(offs[-1])
    r_pad = -(-rows // TILE_N) * TILE_N
    n_tiles = r_pad // TILE_N

    # tile t -> list of (fc-independent) adds: (j, col0, col1, tilecol0)
    tile_adds = [[] for _ in range(n_tiles)]
    for j, w in enumerate(cnt):
        o = int(offs[j])
        t0, t1 = o // TILE_N, (o + w - 1) // TILE_N
        for t in range(t0, t1 + 1):
            s0 = max(0, t * TILE_N - o)
            s1 = min(w, (t + 1) * TILE_N - o)
            tile_adds[t].append((s0, s1, o + s0 - t * TILE_N))

    nc = bacc.Bacc("TRN2", target_bir_lowering=False, debug=False)

    xT = nc.dram_tensor("xT", [DP, r_pad], F32R, kind="ExternalInput").ap()
    w1t = nc.dram_tensor("w1t", [DP, DP], F32R, kind="ExternalInput").ap()
    w2t = nc.dram_tensor("w2t", [DP, DP], F32R, kind="ExternalInput").ap()
    w3t = nc.dram_tensor("w3t", [DP, HP], F32R, kind="ExternalInput").ap()
    w4t = nc.dram_tensor("w4t", [HP, DP], F32R, kind="ExternalInput").ap()
    smw = 2 * NKD + NKH + D + 2 * SC
    smalls = nc.dram_tensor("smalls", [128, smw], F32, kind="ExternalInput").ap()
    zc = nc.dram_tensor("zc", [128, NKD * 256], F32R, kind="ExternalInput").ap()
    y = nc.dram_tensor("y", [SC, D], F32, kind="ExternalOutput").ap()

    relu = mybir.ActivationFunctionType.Relu

    with tile.TileContext(nc) as tc:
        with (
            tc.tile_pool(name="const", bufs=1) as const,
            tc.tile_pool(name="acc", bufs=1) as accp,
            tc.tile_pool(name="xin", bufs=3) as xin,
            tc.tile_pool(name="work", bufs=2) as work,
            tc.tile_pool(name="h2p", bufs=3) as h2p,
            tc.tile_pool(name="psum", bufs=8, space="PSUM") as psum,
        ):
            # ---- constants to SBUF (chunked so first matmuls start early) ----
            w1t_sb = const.tile([128, NKD, DP], F32R)
            w2t_sb = const.tile([128, NKD, DP], F32R)
            w3t_sb = const.tile([128, NKD, HP], F32R)
            w4t_sb = const.tile([128, NKH, DP], F32R)
            for kc in range(NKD):
                nc.sync.dma_start(w1t_sb[:, kc, :], w1t[kc * 128:(kc + 1) * 128, :])
            sm_sb = const.tile([128, smw], F32)
            nc.sync.dma_start(sm_sb[:], smalls)
            o = 0
            b1_sb = sm_sb[:, o:o + NKD]; o += NKD
            b2_sb = sm_sb[:, o:o + NKD]; o += NKD
            b3_sb = sm_sb[:, o:o + NKH]; o += NKH
            b4_sb = sm_sb[:, o:o + D]; o += D
            inv_sb = sm_sb[:, o:o + SC]; o += SC
            beta_sb = sm_sb[:, o:o + SC]; o += SC
            for kc in range(NKD):
                nc.sync.dma_start(w2t_sb[:, kc, :], w2t[kc * 128:(kc + 1) * 128, :])
            for kc in range(NKD):
                nc.sync.dma_start(w3t_sb[:, kc, :], w3t[kc * 128:(kc + 1) * 128, :])
            for kc in range(NKH):
                nc.sync.dma_start(w4t_sb[:, kc, :], w4t[kc * 128:(kc + 1) * 128, :])

            # ---- pooled-sum accumulator ----
            acc = accp.tile([128, NKD, SC], F32)
            nc.any.memset(acc[:], 0.0)

            # ---- main loop over packed row tiles ----
            for t in range(n_tiles):
                n0 = t * TILE_N
                x_t = xin.tile([128, NKD, TILE_N], F32R, tag="x")
                for kc in range(NKD):
                    nc.sync.dma_start(
                        x_t[:, kc, :],
                        xT[kc * 128:(kc + 1) * 128, n0:n0 + TILE_N],
                    )
                h1_t = work.tile([128, NKD, TILE_N], F32R, tag="h1")
                for ec in range(NKD):
                    ps = psum.tile([128, 512], F32, tag="ps")
                    for kc in range(NKD):
                        nc.tensor.matmul(
                            ps[:],
                            w1t_sb[:, kc, ec * 128:(ec + 1) * 128],
                            x_t[:, kc, :],
                            start=(kc == 0), stop=(kc == NKD - 1),
                        )
                    nc.scalar.activation(h1_t[:, ec, :], ps[:], relu,
                                         bias=b1_sb[:, ec:ec + 1])
                for fc in range(NKD):
                    ps = psum.tile([128, 512], F32, tag="ps")
                    for ec in range(NKD):
                        nc.tensor.matmul(
                            ps[:],
                            w2t_sb[:, ec, fc * 128:(fc + 1) * 128],
                            h1_t[:, ec, :],
                            start=(ec == 0), stop=(ec == NKD - 1),
                        )
                    h2_t = h2p.tile([128, TILE_N], F32, tag="h2")
                    nc.scalar.activation(h2_t[:], ps[:], relu,
                                         bias=b2_sb[:, fc:fc + 1])
                    for (s0, s1, tc0) in tile_adds[t]:
                        w = s1 - s0
                        nc.vector.tensor_add(
                            acc[:, fc, s0:s1],
                            acc[:, fc, s0:s1],
                            h2_t[:, tc0:tc0 + w],
                        )

            # ---- c2 = layer2(layer1(0)) on device (exact pad cancellation) ----
            zcol = const.tile([128, NKD, 256], F32R)
            nc.sync.dma_start(zcol[:], zc.rearrange("p (c n) -> p c n", c=NKD))
            h1c = const.tile([128, NKD, 256], F32R)
            c2_sb = const.tile([128, NKD], F32)
            for ec in range(NKD):
                ps = psum.tile([128, 512], F32, tag="ps")
                for kc in range(NKD):
                    nc.tensor.matmul(
                        ps[:, :256],
                        w1t_sb[:, kc, ec * 128:(ec + 1) * 128],
                        zcol[:, kc, :],
                        start=(kc == 0), stop=(kc == NKD - 1),
                    )
                nc.scalar.activation(h1c[:, ec, :], ps[:, :256], relu,
                                     bias=b1_sb[:, ec:ec + 1])
            for fc in range(NKD):
                ps = psum.tile([128, 512], F32, tag="ps")
                for ec in range(NKD):
                    nc.tensor.matmul(
                        ps[:, :256],
                        w2t_sb[:, ec, fc * 128:(fc + 1) * 128],
                        h1c[:, ec, :],
                        start=(ec == 0), stop=(ec == NKD - 1),
                    )
                nc.scalar.activation(c2_sb[:, fc:fc + 1], ps[:, :1], relu,
                                     bias=b2_sb[:, fc:fc + 1])

            # ---- ragged correction: pooled = acc * inv_n - beta * c2 ----
            pooledT = accp.tile([128, NKD, SC], F32R)
            for fc in range(NKD):
                t1 = work.tile([128, SC], F32, tag="t1")
                t2 = work.tile([128, SC], F32, tag="t2")
                nc.vector.tensor_mul(
                    t1[:], beta_sb[:],
                    c2_sb[:, fc:fc + 1].to_broadcast((128, SC)),
                )
                nc.vector.tensor_mul(t2[:], acc[:, fc, :], inv_sb[:])
                nc.vector.tensor_sub(pooledT[:, fc, :], t2[:], t1[:])

            # ---- layer 3: h3T = relu(W3 pooled + b3), [HP, SC] ----
            h3T = accp.tile([128, NKH, SC], F32R)
            for hc in range(NKH):
                ps = psum.tile([128, 512], F32, tag="ps")
                for kc in range(NKD):
                    nc.tensor.matmul(
                        ps[:],
                        w3t_sb[:, kc, hc * 128:(hc + 1) * 128],
                        pooledT[:, kc, :],
                        start=(kc == 0), stop=(kc == NKD - 1),
                    )
                nc.scalar.activation(h3T[:, hc, :], ps[:], relu,
                                     bias=b3_sb[:, hc:hc + 1])

            # ---- layer 4 (flipped): y[s, :] = relu(h3T.T @ W4T + b4) ----
            for sc4 in range(SC // 128):
                ps = psum.tile([128, 512], F32, tag="ps")
                for hc in range(NKH):
                    nc.tensor.matmul(
                        ps[:, :D],
                        h3T[:, hc, sc4 * 128:(sc4 + 1) * 128],
                        w4t_sb[:, hc, :D],
                        start=(hc == 0), stop=(hc == NKH - 1),
                    )
                y_t = work.tile([128, D], F32, tag="yt")
                nc.vector.tensor_add(y_t[:], ps[:, :D], b4_sb[:])
                nc.vector.tensor_scalar_max(y_t[:], y_t[:], 0.0)
                nc.sync.dma_start(y[sc4 * 128:(sc4 + 1) * 128, :], y_t[:])

    nc.compile()
    return nc, rows, r_pad


def kernel(**inputs):
    global LAST_EXEC_NS
    all_x = np.asarray(inputs["all_x"], dtype=np.float32)
    numInputs = np.asarray(inputs["numInputs"]).astype(np.int64)
    W1 = np.asarray(inputs["W1"], dtype=np.float32)
    b1 = np.asarray(inputs["b1"], dtype=np.float32)
    W2 = np.asarray(inputs["W2"], dtype=np.float32)
    b2 = np.asarray(inputs["b2"], dtype=np.float32)
    W3 = np.asarray(inputs["W3"], dtype=np.float32)
    b3 = np.asarray(inputs["b3"], dtype=np.float32)
    W4 = np.asarray(inputs["W4"], dtype=np.float32)
    b4 = np.asarray(inputs["b4"], dtype=np.float32)

    # ---- shared ragged geometry: sort by n desc, deal round-robin ----
    perm = np.argsort(-numInputs, kind="stable")      # global rank -> sample
    G = np.array([(numInputs > j).sum() for j in range(M)])  # plane sizes
    cnt = np.ceil(G / N_CORES).astype(np.int64)       # shared per-core widths
    cnt_key = tuple(int(c) for c in cnt)
    B = np.array([(cnt > s).sum() for s in range(SC)], np.int64)  # slots/column

    cached = _PROGRAM_CACHE.get(cnt_key)
    if cached is None:
        cached = _build_program(cnt_key)
        _PROGRAM_CACHE[cnt_key] = cached
    nc, rows, r_pad = cached

    def padded_T(w, rows_to, cols_to=None):
        out_dim, in_dim = w.shape
        cols_to = cols_to or out_dim
        a = np.zeros((rows_to, cols_to), np.float32)
        a[:in_dim, :out_dim] = w.T
        return a

    w1t = padded_T(W1, DP, DP)
    w2t = padded_T(W2, DP, DP)
    w3t = padded_T(W3, DP, HP)
    w4t = padded_T(W4, HP, DP)

    def chunked_bias(b, pad_to, nchunks):
        a = np.zeros(pad_to, np.float32)
        a[:b.shape[0]] = b
        return np.ascontiguousarray(a.reshape(nchunks, 128).T)

    b1c = chunked_bias(b1, DP, NKD)
    b2c = chunked_bias(b2, DP, NKD)
    b3c = chunked_bias(b3, HP, NKH)
    b4rep = np.ascontiguousarray(np.broadcast_to(b4[None, :], (128, D)))

    x_flat = all_x.reshape(S * M, D)
    x_flat = np.vstack([x_flat, np.zeros((1, D), np.float32)])  # row -1 = zeros
    cnz = [j for j in range(M) if cnt[j] > 0]

    in_maps = []
    for c in range(N_CORES):
        # build packed row index list for this core
        idx_parts = []
        for j in cnz:
            w = int(cnt[j])
            scols = np.arange(w)
            ranks = N_CORES * scols + c
            valid = ranks < G[j]
            samp = np.where(valid, perm[np.minimum(ranks, S - 1)], -1)
            fi = np.where(valid, samp * M + j, S * M)  # S*M -> zero row
            idx_parts.append(fi)
        idx = np.concatenate(idx_parts)
        xp = x_flat[idx]                               # [rows, D]
        xT = np.zeros((DP, r_pad), np.float32)
        xT[:D, :rows] = xp.T

        ranks_c = N_CORES * np.arange(SC) + c
        n_col = numInputs[perm[ranks_c]].astype(np.float32)
        inv = (1.0 / n_col).astype(np.float32)
        beta = ((B - n_col) / n_col).astype(np.float32)
        smalls = np.concatenate([
            b1c, b2c, b3c, b4rep,
            np.broadcast_to(inv[None, :], (128, SC)),
            np.broadcast_to(beta[None, :], (128, SC)),
        ], axis=1).astype(np.float32)
        in_maps.append({
            "xT": xT,
            "w1t": w1t, "w2t": w2t, "w3t": w3t, "w4t": w4t,
            "zc": np.zeros((128, NKD * 256), np.float32),
            "smalls": np.ascontiguousarray(smalls),
        })

    res = run_bass_kernel_spmd(nc, in_maps, core_ids=list(range(N_CORES)))
    LAST_EXEC_NS = res.exec_time_ns

    out = np.empty((S, D), np.float32)
    for c in range(N_CORES):
        ranks_c = N_CORES * np.arange(SC) + c
        out[perm[ranks_c]] = res.results[c]["y"]
    return out[:, None, :].astype(np.float32)


# revision 9
# speedup vs baseline: 1.7121x; 1.0527x over previous
"""Trainium2 Bass kernel for nn_Dereverber_v3 (ragged MLP + masked mean-pool + MLP).

Strategy (pure data parallelism over 8 NeuronCores, ragged-packed):
- Sort samples by numInputs descending, deal round-robin to the 8 cores so all
  cores share one compile-time row geometry (per-core padding <= 32 rows).
- Pack valid (sample, slot) rows slot-major into "planes": plane j holds
  column s for every sample rank s with numInputs > j; plane widths cnt_j are
  a shared prefix staircase. Invalid/pad rows are zeroed.
- Device per core: h1 = relu(W1 xT + b1), h2 = relu(W2 h1 + b2) over packed
  rows only (~52% of dense), activations transposed [feat, rows].
- Mean-pool: acc[f, s] = sum over planes of h2 columns (DVE adds over the
  plane staircase); pad columns contribute c2 = MLP2(0), computed on device
  from a zero column, cancelled exactly: pooled = (acc - (B_s - n_s) c2)/n_s.
- h3 = relu(W3 pooled + b3); y = relu(W4 h3 + b4) with the last matmul flipped
  so output lands sample-major; host undoes the sort/deal permutation.
- All matmuls in float32r (full PE rate, ~1e-4 rel err).
"""

import numpy as np
import concourse.bass as bass  # noqa: F401
import concourse.mybir as mybir
import concourse.tile as tile
from concourse import bacc
from concourse.bass_utils import run_bass_kernel_spmd

S, M, D, H = 4096, 32, 420, 840
N_CORES = 8
SC = S // N_CORES            # samples per core (512)
TILE_N = 512                 # rows per tile
DP = 512                     # padded feature dim (d, e, f): 4 chunks of 128
HP = 896                     # padded hidden dim (h): 7 chunks of 128
NKD = DP // 128              # 4
NKH = HP // 128              # 7

F32 = mybir.dt.float32
F32R = mybir.dt.float32r

LAST_EXEC_NS = None
_PROGRAM_CACHE = {}


def _build_program(cnt):
    """cnt: tuple of plane widths (len M, non-increasing). Shared across cores."""
    cnt = [c for c in cnt if c > 0]
    offs = np.concatenate([[0], np.cumsum(cnt)])  # plane start offsets
    rows = int(offs[-1])
    r_pad = -(-rows // TILE_N) * TILE_N
    n_tiles = r_pad // TILE_N

    # tile t -> list of (fc-independent) adds: (j, col0, col1, tilecol0)
    tile_adds = [[] for _ in range(n_tiles)]
    for j, w in enumerate(cnt):
        o = int(offs[j])
        t0, t1 = o // TILE_N, (o + w - 1) // TILE_N
        for t in range(t0, t1 + 1):
            s0 = max(0, t * TILE_N - o)
            s1 = min(w, (t + 1) * TILE_N - o)
            tile_adds[t].append((s0, s1, o + s0 - t * TILE_N))

    nc = bacc.Bacc("TRN2", target_bir_lowering=False, debug=False)

    xT = nc.dram_tensor("xT", [DP, r_pad], F32R, kind="ExternalInput").ap()
    w1t = nc.dram_tensor("w1t", [DP, DP], F32R, kind="ExternalInput").ap()
    w2t = nc.dram_tensor("w2t", [DP, DP], F32R, kind="ExternalInput").ap()
    w3t = nc.dram_tensor("w3t", [DP, HP], F32R, kind="ExternalInput").ap()
    w4t = nc.dram_tensor("w4t", [HP, DP], F32R, kind="ExternalInput").ap()
    smw = 2 * NKD + NKH + D + 2 * SC
    smalls = nc.dram_tensor("smalls", [128, smw], F32, kind="ExternalInput").ap()
    zc = nc.dram_tensor("zc", [128, NKD * 256], F32R, kind="ExternalInput").ap()
    y = nc.dram_tensor("y", [SC, D], F32, kind="ExternalOutput").ap()

    relu = mybir.ActivationFunctionType.Relu

    with tile.TileContext(nc) as tc:
        with (
            tc.tile_pool(name="const", bufs=1) as const,
            tc.tile_pool(name="acc", bufs=1) as accp,
            tc.tile_pool(name="xin", bufs=3) as xin,
            tc.tile_pool(name="work", bufs=2) as work,
            tc.tile_pool(name="h2p", bufs=3) as h2p,
            tc.tile_pool(name="psum", bufs=8, space="PSUM") as psum,
        ):
            # ---- constants to SBUF via the Scalar HWDGE queue so the Sync
            # queue is free for x tiles (DMA issues serialize per queue) ----
            w1t_sb = const.tile([128, NKD, DP], F32R)
            w2t_sb = const.tile([128, NKD, DP], F32R)
            w3t_sb = const.tile([128, NKD, HP], F32R)
            w4t_sb = const.tile([128, NKH, DP], F32R)
            nc.scalar.dma_start(w1t_sb[:], w1t.rearrange("(c p) e -> p c e", p=128))
            sm_sb = const.tile([128, smw], F32)
            nc.scalar.dma_start(sm_sb[:], smalls)
            o = 0
            b1_sb = sm_sb[:, o:o + NKD]; o += NKD
            b2_sb = sm_sb[:, o:o + NKD]; o += NKD
            b3_sb = sm_sb[:, o:o + NKH]; o += NKH
            b4_sb = sm_sb[:, o:o + D]; o += D
            inv_sb = sm_sb[:, o:o + SC]; o += SC
            beta_sb = sm_sb[:, o:o + SC]; o += SC
            nc.scalar.dma_start(w2t_sb[:], w2t.rearrange("(c p) e -> p c e", p=128))
            nc.scalar.dma_start(w3t_sb[:], w3t.rearrange("(c p) e -> p c e", p=128))
            nc.scalar.dma_start(w4t_sb[:], w4t.rearrange("(c p) e -> p c e", p=128))

            # ---- pooled-sum accumulator ----
            acc = accp.tile([128, NKD, SC], F32)
            nc.any.memset(acc[:], 0.0)

            # ---- main loop over packed row tiles ----
            for t in range(n_tiles):
                n0 = t * TILE_N
                x_t = xin.tile([128, NKD, TILE_N], F32R, tag="x")
                nc.sync.dma_start(
                    x_t[:], xT[:, n0:n0 + TILE_N].rearrange("(c p) n -> p c n", p=128)
                )
                h1_t = work.tile([128, NKD, TILE_N], F32R, tag="h1")
                for ec in range(NKD):
                    ps = psum.tile([128, 512], F32, tag="ps")
                    for kc in range(NKD):
                        nc.tensor.matmul(
                            ps[:],
                            w1t_sb[:, kc, ec * 128:(ec + 1) * 128],
                            x_t[:, kc, :],
                            start=(kc == 0), stop=(kc == NKD - 1),
                        )
                    nc.scalar.activation(h1_t[:, ec, :], ps[:], relu,
                                         bias=b1_sb[:, ec:ec + 1])
                for fc in range(NKD):
                    ps = psum.tile([128, 512], F32, tag="ps")
                    for ec in range(NKD):
                        nc.tensor.matmul(
                            ps[:],
                            w2t_sb[:, ec, fc * 128:(fc + 1) * 128],
                            h1_t[:, ec, :],
                            start=(ec == 0), stop=(ec == NKD - 1),
                        )
                    h2_t = h2p.tile([128, TILE_N], F32, tag="h2")
                    nc.scalar.activation(h2_t[:], ps[:], relu,
                                         bias=b2_sb[:, fc:fc + 1])
                    for (s0, s1, tc0) in tile_adds[t]:
                        w = s1 - s0
                        nc.vector.tensor_add(
                            acc[:, fc, s0:s1],
                            acc[:, fc, s0:s1],
                            h2_t[:, tc0:tc0 + w],
                        )

            # ---- c2 = layer2(layer1(0)) on device (exact pad cancellation) ----
            zcol = const.tile([128, NKD, 256], F32R)
            nc.scalar.dma_start(zcol[:], zc.rearrange("p (c n) -> p c n", c=NKD))
            h1c = const.tile([128, NKD, 256], F32R)
            c2_sb = const.tile([128, NKD], F32)
            for ec in range(NKD):
                ps = psum.tile([128, 512], F32, tag="ps")
                for kc in range(NKD):
                    nc.tensor.matmul(
                        ps[:, :256],
                        w1t_sb[:, kc, ec * 128:(ec + 1) * 128],
                        zcol[:, kc, :],
                        start=(kc == 0), stop=(kc == NKD - 1),
                    )
                nc.scalar.activation(h1c[:, ec, :], ps[:, :256], relu,
                                     bias=b1_sb[:, ec:ec + 1])
            for fc in range(NKD):
                ps = psum.tile([128, 512], F32, tag="ps")
                for ec in range(NKD):
                    nc.tensor.matmul(
                        ps[:, :256],
                        w2t_sb[:, ec, fc * 128:(fc + 1) * 128],
                        h1c[:, ec, :],
                        start=(ec == 0), stop=(ec == NKD - 1),
                    )
                nc.scalar.activation(c2_sb[:, fc:fc + 1], ps[:, :1], relu,
                                     bias=b2_sb[:, fc:fc + 1])

            # ---- ragged correction: pooled = acc * inv_n - beta * c2 ----
            pooledT = accp.tile([128, NKD, SC], F32R)
            for fc in range(NKD):
                t1 = work.tile([128, SC], F32, tag="t1")
                t2 = work.tile([128, SC], F32, tag="t2")
                nc.vector.tensor_mul(
                    t1[:], beta_sb[:],
                    c2_sb[:, fc:fc + 1].to_broadcast((128, SC)),
                )
                nc.vector.tensor_mul(t2[:], acc[:, fc, :], inv_sb[:])
                nc.vector.tensor_sub(pooledT[:, fc, :], t2[:], t1[:])

            # ---- layer 3: h3T = relu(W3 pooled + b3), [HP, SC] ----
            h3T = accp.tile([128, NKH, SC], F32R)
            for hc in range(NKH):
                ps = psum.tile([128, 512], F32, tag="ps")
                for kc in range(NKD):
                    nc.tensor.matmul(
                        ps[:],
                        w3t_sb[:, kc, hc * 128:(hc + 1) * 128],
                        pooledT[:, kc, :],
                        start=(kc == 0), stop=(kc == NKD - 1),
                    )
                nc.scalar.activation(h3T[:, hc, :], ps[:], relu,
                                     bias=b3_sb[:, hc:hc + 1])

            # ---- layer 4 (flipped): y[s, :] = relu(h3T.T @ W4T + b4) ----
            for sc4 in range(SC // 128):
                ps = psum.tile([128, 512], F32, tag="ps")
                for hc in range(NKH):
                    nc.tensor.matmul(
                        ps[:, :D],
                        h3T[:, hc, sc4 * 128:(sc4 + 1) * 128],
                        w4t_sb[:, hc, :D],
                        start=(hc == 0), stop=(hc == NKH - 1),
                    )
                y_t = work.tile([128, D], F32, tag="yt")
                nc.vector.tensor_add(y_t[:], ps[:, :D], b4_sb[:])
                nc.vector.tensor_scalar_max(y_t[:], y_t[:], 0.0)
                nc.sync.dma_start(y[sc4 * 128:(sc4 + 1) * 128, :], y_t[:])

    nc.compile()
    return nc, rows, r_pad


def kernel(**inputs):
    global LAST_EXEC_NS
    all_x = np.asarray(inputs["all_x"], dtype=np.float32)
    numInputs = np.asarray(inputs["numInputs"]).astype(np.int64)
    W1 = np.asarray(inputs["W1"], dtype=np.float32)
    b1 = np.asarray(inputs["b1"], dtype=np.float32)
    W2 = np.asarray(inputs["W2"], dtype=np.float32)
    b2 = np.asarray(inputs["b2"], dtype=np.float32)
    W3 = np.asarray(inputs["W3"], dtype=np.float32)
    b3 = np.asarray(inputs["b3"], dtype=np.float32)
    W4 = np.asarray(inputs["W4"], dtype=np.float32)
    b4 = np.asarray(inputs["b4"], dtype=np.float32)

    # ---- shared ragged geometry: sort by n desc, deal round-robin ----
    perm = np.argsort(-numInputs, kind="stable")      # global rank -> sample
    G = np.array([(numInputs > j).sum() for j in range(M)])  # plane sizes
    cnt = np.ceil(G / N_CORES).astype(np.int64)       # shared per-core widths
    cnt_key = tuple(int(c) for c in cnt)
    B = np.array([(cnt > s).sum() for s in range(SC)], np.int64)  # slots/column

    cached = _PROGRAM_CACHE.get(cnt_key)
    if cached is None:
        cached = _build_program(cnt_key)
        _PROGRAM_CACHE[cnt_key] = cached
    nc, rows, r_pad = cached

    def padded_T(w, rows_to, cols_to=None):
        out_dim, in_dim = w.shape
        cols_to = cols_to or out_dim
        a = np.zeros((rows_to, cols_to), np.float32)
        a[:in_dim, :out_dim] = w.T
        return a

    w1t = padded_T(W1, DP, DP)
    w2t = padded_T(W2, DP, DP)
    w3t = padded_T(W3, DP, HP)
    w4t = padded_T(W4, HP, DP)

    def chunked_bias(b, pad_to, nchunks):
        a = np.zeros(pad_to, np.float32)
        a[:b.shape[0]] = b
        return np.ascontiguousarray(a.reshape(nchunks, 128).T)

    b1c = chunked_bias(b1, DP, NKD)
    b2c = chunked_bias(b2, DP, NKD)
    b3c = chunked_bias(b3, HP, NKH)
    b4rep = np.ascontiguousarray(np.broadcast_to(b4[None, :], (128, D)))

    x_flat = all_x.reshape(S * M, D)
    x_flat = np.vstack([x_flat, np.zeros((1, D), np.float32)])  # row -1 = zeros
    cnz = [j for j in range(M) if cnt[j] > 0]

    in_maps = []
    for c in range(N_CORES):
        # build packed row index list for this core
        idx_parts = []
        for j in cnz:
            w = int(cnt[j])
            scols = np.arange(w)
            ranks = N_CORES * scols + c
            valid = ranks < G[j]
            samp = np.where(valid, perm[np.minimum(ranks, S - 1)], -1)
            fi = np.where(valid, samp * M + j, S * M)  # S*M -> zero row
            idx_parts.append(fi)
        idx = np.concatenate(idx_parts)
        xp = x_flat[idx]                               # [rows, D]
        xT = np.zeros((DP, r_pad), np.float32)
        xT[:D, :rows] = xp.T

        ranks_c = N_CORES * np.arange(SC) + c
        n_col = numInputs[perm[ranks_c]].astype(np.float32)
        inv = (1.0 / n_col).astype(np.float32)
        beta = ((B - n_col) / n_col).astype(np.float32)
        smalls = np.concatenate([
            b1c, b2c, b3c, b4rep,
            np.broadcast_to(inv[None, :], (128, SC)),
            np.broadcast_to(beta[None, :], (128, SC)),
        ], axis=1).astype(np.float32)
        in_maps.append({
            "xT": xT,
            "w1t": w1t, "w2t": w2t, "w3t": w3t, "w4t": w4t,
            "zc": np.zeros((128, NKD * 256), np.float32),
            "smalls": np.ascontiguousarray(smalls),
        })

    res = run_bass_kernel_spmd(nc, in_maps, core_ids=list(range(N_CORES)))
    LAST_EXEC_NS = res.exec_time_ns

    out = np.empty((S, D), np.float32)
    for c in range(N_CORES):
        ranks_c = N_CORES * np.arange(SC) + c
        out[perm[ranks_c]] = res.results[c]["y"]
    return out[:, None, :].astype(np.float32)


# revision 11
# speedup vs baseline: 1.7410x; 1.0169x over previous
"""Trainium2 Bass kernel for nn_Dereverber_v3 (ragged MLP + masked mean-pool + MLP).

Strategy (pure data parallelism over 8 NeuronCores, ragged-packed):
- Sort samples by numInputs descending, deal round-robin to the 8 cores so all
  cores share one compile-time row geometry (per-core padding <= 32 rows).
- Pack valid (sample, slot) rows slot-major into "planes": plane j holds
  column s for every sample rank s with numInputs > j; plane widths cnt_j are
  a shared prefix staircase. Invalid/pad rows are zeroed.
- Device per core: h1 = relu(W1 xT + b1), h2 = relu(W2 h1 + b2) over packed
  rows only (~52% of dense), activations transposed [feat, rows].
- Mean-pool: acc[f, s] = sum over planes of h2 columns (DVE adds over the
  plane staircase); pad columns contribute c2 = MLP2(0), computed on device
  from a zero column, cancelled exactly: pooled = (acc - (B_s - n_s) c2)/n_s.
- h3 = relu(W3 pooled + b3); y = relu(W4 h3 + b4) with the last matmul flipped
  so output lands sample-major; host undoes the sort/deal permutation.
- All matmuls in float32r (full PE rate, ~1e-4 rel err).
"""

import numpy as np
import concourse.bass as bass  # noqa: F401
import concourse.mybir as mybir
import concourse.tile as tile
from concourse import bacc
from concourse.bass_utils import run_bass_kernel_spmd

S, M, D, H = 4096, 32, 420, 840
N_CORES = 8
SC = S // N_CORES            # samples per core (512)
TILE_N = 512                 # rows per tile
DP = 512                     # padded feature dim (d, e, f): 4 chunks of 128
HP = 896                     # padded hidden dim (h): 7 chunks of 128
NKD = DP // 128              # 4
NKH = HP // 128              # 7

F32 = mybir.dt.float32
F32R = mybir.dt.float32r

LAST_EXEC_NS = None
_PROGRAM_CACHE = {}


def _build_program(cnt):
    """cnt: tuple of plane widths (len M, non-increasing). Shared across cores."""
    cnt = [c for c in cnt if c > 0]
    offs = np.concatenate([[0], np.cumsum(cnt)])  # plane start offsets
    rows = int(offs[-1])
    r_pad = -(-rows // TILE_N) * TILE_N
    n_tiles = r_pad // TILE_N

    # tile t -> list of (fc-independent) adds: (j, col0, col1, tilecol0)
    tile_adds = [[] for _ in range(n_tiles)]
    for j, w in enumerate(cnt):
        o = int(offs[j])
        t0, t1 = o // TILE_N, (o + w - 1) // TILE_N
        for t in range(t0, t1 + 1):
            s0 = max(0, t * TILE_N - o)
            s1 = min(w, (t + 1) * TILE_N - o)
            tile_adds[t].append((s0, s1, o + s0 - t * TILE_N))

    nc = bacc.Bacc("TRN2", target_bir_lowering=False, debug=False)

    xT = nc.dram_tensor("xT", [DP, r_pad], F32R, kind="ExternalInput").ap()
    w1t = nc.dram_tensor("w1t", [DP, DP], F32R, kind="ExternalInput").ap()
    w2t = nc.dram_tensor("w2t", [DP, DP], F32R, kind="ExternalInput").ap()
    w3t = nc.dram_tensor("w3t", [DP, HP], F32R, kind="ExternalInput").ap()
    w4t = nc.dram_tensor("w4t", [HP, DP], F32R, kind="ExternalInput").ap()
    smw = 2 * NKD + NKH + D + 2 * SC
    smalls = nc.dram_tensor("smalls", [128, smw], F32, kind="ExternalInput").ap()
    zc = nc.dram_tensor("zc", [128, NKD * 256], F32R, kind="ExternalInput").ap()
    y = nc.dram_tensor("y", [SC, D], F32, kind="ExternalOutput").ap()

    relu = mybir.ActivationFunctionType.Relu

    with tile.TileContext(nc) as tc:
        with (
            tc.tile_pool(name="const", bufs=1) as const,
            tc.tile_pool(name="acc", bufs=1) as accp,
            tc.tile_pool(name="xin", bufs=3) as xin,
            tc.tile_pool(name="work", bufs=2) as work,
            tc.tile_pool(name="h2p", bufs=3) as h2p,
            tc.tile_pool(name="psum", bufs=8, space="PSUM") as psum,
        ):
            # ---- constants to SBUF via the Scalar HWDGE queue so the Sync
            # queue is free for x tiles (DMA issues serialize per queue) ----
            w1t_sb = const.tile([128, NKD, DP], F32R)
            w2t_sb = const.tile([128, NKD, DP], F32R)
            w3t_sb = const.tile([128, NKD, HP], F32R)
            w4t_sb = const.tile([128, NKH, DP], F32R)
            nc.scalar.dma_start(w1t_sb[:], w1t.rearrange("(c p) e -> p c e", p=128))
            sm_sb = const.tile([128, smw], F32)
            nc.scalar.dma_start(sm_sb[:], smalls)
            o = 0
            b1_sb = sm_sb[:, o:o + NKD]; o += NKD
            b2_sb = sm_sb[:, o:o + NKD]; o += NKD
            b3_sb = sm_sb[:, o:o + NKH]; o += NKH
            b4_sb = sm_sb[:, o:o + D]; o += D
            inv_sb = sm_sb[:, o:o + SC]; o += SC
            beta_sb = sm_sb[:, o:o + SC]; o += SC
            nc.scalar.dma_start(w2t_sb[:], w2t.rearrange("(c p) e -> p c e", p=128))
            nc.scalar.dma_start(w3t_sb[:], w3t.rearrange("(c p) e -> p c e", p=128))
            nc.scalar.dma_start(w4t_sb[:], w4t.rearrange("(c p) e -> p c e", p=128))

            # ---- PE warm-up: dummy bf16 matmuls bridge the DMA head so the
            # HAM clock-gate is at 8/8 when real work arrives ----
            wz = const.tile([128, 512], mybir.dt.bfloat16)
            nc.vector.memset(wz[:], 0.0)
            for wi in range(40):
                wps = psum.tile([128, 512], F32, tag="ps")
                nc.tensor.matmul(wps[:], wz[:, :128], wz[:],
                                 start=True, stop=True)

            # ---- pooled-sum accumulator ----
            acc = accp.tile([128, NKD, SC], F32)
            nc.any.memset(acc[:], 0.0)

            # ---- main loop over packed row tiles ----
            for t in range(n_tiles):
                n0 = t * TILE_N
                tn = min(TILE_N, max(rows - n0, 256))
                x_t = xin.tile([128, NKD, TILE_N], F32R, tag="x")
                nc.sync.dma_start(
                    x_t[:, :, :tn],
                    xT[:, n0:n0 + tn].rearrange("(c p) n -> p c n", p=128),
                )
                h1_t = work.tile([128, NKD, TILE_N], F32R, tag="h1")
                for ec in range(NKD):
                    ps = psum.tile([128, 512], F32, tag="ps")
                    for kc in range(NKD):
                        nc.tensor.matmul(
                            ps[:, :tn],
                            w1t_sb[:, kc, ec * 128:(ec + 1) * 128],
                            x_t[:, kc, :tn],
                            start=(kc == 0), stop=(kc == NKD - 1),
                        )
                    nc.scalar.activation(h1_t[:, ec, :tn], ps[:, :tn], relu,
                                         bias=b1_sb[:, ec:ec + 1])
                for fc in range(NKD):
                    ps = psum.tile([128, 512], F32, tag="ps")
                    for ec in range(NKD):
                        nc.tensor.matmul(
                            ps[:, :tn],
                            w2t_sb[:, ec, fc * 128:(fc + 1) * 128],
                            h1_t[:, ec, :tn],
                            start=(ec == 0), stop=(ec == NKD - 1),
                        )
                    h2_t = h2p.tile([128, TILE_N], F32, tag="h2")
                    nc.scalar.activation(h2_t[:, :tn], ps[:, :tn], relu,
                                         bias=b2_sb[:, fc:fc + 1])
                    for (s0, s1, tc0) in tile_adds[t]:
                        w = s1 - s0
                        nc.vector.tensor_add(
                            acc[:, fc, s0:s1],
                            acc[:, fc, s0:s1],
                            h2_t[:, tc0:tc0 + w],
                        )

            # ---- c2 = layer2(layer1(0)) on device (exact pad cancellation) ----
            zcol = const.tile([128, NKD, 256], F32R)
            nc.scalar.dma_start(zcol[:], zc.rearrange("p (c n) -> p c n", c=NKD))
            h1c = const.tile([128, NKD, 256], F32R)
            c2_sb = const.tile([128, NKD], F32)
            for ec in range(NKD):
                ps = psum.tile([128, 512], F32, tag="ps")
                for kc in range(NKD):
                    nc.tensor.matmul(
                        ps[:, :256],
                        w1t_sb[:, kc, ec * 128:(ec + 1) * 128],
                        zcol[:, kc, :],
                        start=(kc == 0), stop=(kc == NKD - 1),
                    )
                nc.scalar.activation(h1c[:, ec, :], ps[:, :256], relu,
                                     bias=b1_sb[:, ec:ec + 1])
            for fc in range(NKD):
                ps = psum.tile([128, 512], F32, tag="ps")
                for ec in range(NKD):
                    nc.tensor.matmul(
                        ps[:, :256],
                        w2t_sb[:, ec, fc * 128:(fc + 1) * 128],
                        h1c[:, ec, :],
                        start=(ec == 0), stop=(ec == NKD - 1),
                    )
                nc.scalar.activation(c2_sb[:, fc:fc + 1], ps[:, :1], relu,
                                     bias=b2_sb[:, fc:fc + 1])

            # ---- ragged correction: pooled = acc * inv_n - beta * c2 ----
            pooledT = accp.tile([128, NKD, SC], F32R)
            for fc in range(NKD):
                t1 = work.tile([128, SC], F32, tag="t1")
                t2 = work.tile([128, SC], F32, tag="t2")
                nc.vector.tensor_mul(
                    t1[:], beta_sb[:],
                    c2_sb[:, fc:fc + 1].to_broadcast((128, SC)),
                )
                nc.vector.tensor_mul(t2[:], acc[:, fc, :], inv_sb[:])
                nc.vector.tensor_sub(pooledT[:, fc, :], t2[:], t1[:])

            # ---- layer 3: h3T = relu(W3 pooled + b3), [HP, SC] ----
            h3T = accp.tile([128, NKH, SC], F32R)
            for hc in range(NKH):
                ps = psum.tile([128, 512], F32, tag="ps")
                for kc in range(NKD):
                    nc.tensor.matmul(
                        ps[:],
                        w3t_sb[:, kc, hc * 128:(hc + 1) * 128],
                        pooledT[:, kc, :],
                        start=(kc == 0), stop=(kc == NKD - 1),
                    )
                nc.scalar.activation(h3T[:, hc, :], ps[:], relu,
                                     bias=b3_sb[:, hc:hc + 1])

            # ---- layer 4 (flipped): y[s, :] = relu(h3T.T @ W4T + b4) ----
            for sc4 in range(SC // 128):
                ps = psum.tile([128, 512], F32, tag="ps")
                for hc in range(NKH):
                    nc.tensor.matmul(
                        ps[:, :D],
                        h3T[:, hc, sc4 * 128:(sc4 + 1) * 128],
                        w4t_sb[:, hc, :D],
                        start=(hc == 0), stop=(hc == NKH - 1),
                    )
                y_t = work.tile([128, D], F32, tag="yt")
                nc.vector.tensor_add(y_t[:], ps[:, :D], b4_sb[:])
                nc.vector.tensor_scalar_max(y_t[:], y_t[:], 0.0)
                nc.sync.dma_start(y[sc4 * 128:(sc4 + 1) * 128, :], y_t[:])

    nc.compile()
    return nc, rows, r_pad


def kernel(**inputs):
    global LAST_EXEC_NS
    all_x = np.asarray(inputs["all_x"], dtype=np.float32)
    numInputs = np.asarray(inputs["numInputs"]).astype(np.int64)
    W1 = np.asarray(inputs["W1"], dtype=np.float32)
    b1 = np.asarray(inputs["b1"], dtype=np.float32)
    W2 = np.asarray(inputs["W2"], dtype=np.float32)
    b2 = np.asarray(inputs["b2"], dtype=np.float32)
    W3 = np.asarray(inputs["W3"], dtype=np.float32)
    b3 = np.asarray(inputs["b3"], dtype=np.float32)
    W4 = np.asarray(inputs["W4"], dtype=np.float32)
    b4 = np.asarray(inputs["b4"], dtype=np.float32)

    # ---- shared ragged geometry: sort by n desc, deal round-robin ----
    perm = np.argsort(-numInputs, kind="stable")      # global rank -> sample
    G = np.array([(numInputs > j).sum() for j in range(M)])  # plane sizes
    cnt = np.ceil(G / N_CORES).astype(np.int64)       # shared per-core widths
    cnt_key = tuple(int(c) for c in cnt)
    B = np.array([(cnt > s).sum() for s in range(SC)], np.int64)  # slots/column

    cached = _PROGRAM_CACHE.get(cnt_key)
    if cached is None:
        cached = _build_program(cnt_key)
        _PROGRAM_CACHE[cnt_key] = cached
    nc, rows, r_pad = cached

    def padded_T(w, rows_to, cols_to=None):
        out_dim, in_dim = w.shape
        cols_to = cols_to or out_dim
        a = np.zeros((rows_to, cols_to), np.float32)
        a[:in_dim, :out_dim] = w.T
        return a

    w1t = padded_T(W1, DP, DP)
    w2t = padded_T(W2, DP, DP)
    w3t = padded_T(W3, DP, HP)
    w4t = padded_T(W4, HP, DP)

    def chunked_bias(b, pad_to, nchunks):
        a = np.zeros(pad_to, np.float32)
        a[:b.shape[0]] = b
        return np.ascontiguousarray(a.reshape(nchunks, 128).T)

    b1c = chunked_bias(b1, DP, NKD)
    b2c = chunked_bias(b2, DP, NKD)
    b3c = chunked_bias(b3, HP, NKH)
    b4rep = np.ascontiguousarray(np.broadcast_to(b4[None, :], (128, D)))

    x_flat = all_x.reshape(S * M, D)
    x_flat = np.vstack([x_flat, np.zeros((1, D), np.float32)])  # row -1 = zeros
    cnz = [j for j in range(M) if cnt[j] > 0]

    in_maps = []
    for c in range(N_CORES):
        # build packed row index list for this core
        idx_parts = []
        for j in cnz:
            w = int(cnt[j])
            scols = np.arange(w)
            ranks = N_CORES * scols + c
            valid = ranks < G[j]
            samp = np.where(valid, perm[np.minimum(ranks, S - 1)], -1)
            fi = np.where(valid, samp * M + j, S * M)  # S*M -> zero row
            idx_parts.append(fi)
        idx = np.concatenate(idx_parts)
        xp = x_flat[idx]                               # [rows, D]
        xT = np.zeros((DP, r_pad), np.float32)
        xT[:D, :rows] = xp.T

        ranks_c = N_CORES * np.arange(SC) + c
        n_col = numInputs[perm[ranks_c]].astype(np.float32)
        inv = (1.0 / n_col).astype(np.float32)
        beta = ((B - n_col) / n_col).astype(np.float32)
        smalls = np.concatenate([
            b1c, b2c, b3c, b4rep,
            np.broadcast_to(inv[None, :], (128, SC)),
            np.broadcast_to(beta[None, :], (128, SC)),
        ], axis=1).astype(np.float32)
        in_maps.append({
            "xT": xT,
            "w1t": w1t, "w2t": w2t, "w3t": w3t, "w4t": w4t,
            "zc": np.zeros((128, NKD * 256), np.float32),
            "smalls": np.ascontiguousarray(smalls),
        })

    res = run_bass_kernel_spmd(nc, in_maps, core_ids=list(range(N_CORES)))
    LAST_EXEC_NS = res.exec_time_ns

    out = np.empty((S, D), np.float32)
    for c in range(N_CORES):
        ranks_c = N_CORES * np.arange(SC) + c
        out[perm[ranks_c]] = res.results[c]["y"]
    return out[:, None, :].astype(np.float32)


# revision 22
# speedup vs baseline: 1.7910x; 1.0287x over previous
"""Trainium2 Bass kernel for nn_Dereverber_v3 (ragged MLP + masked mean-pool + MLP).

Strategy (pure data parallelism over 8 NeuronCores, ragged-packed):
- Sort samples by numInputs descending, deal round-robin to the 8 cores so all
  cores share one compile-time row geometry (per-core padding <= 32 rows).
- Pack valid (sample, slot) rows slot-major into "planes": plane j holds
  column s for every sample rank s with numInputs > j; plane widths cnt_j are
  a shared prefix staircase. Invalid/pad rows are zeroed.
- Device per core: h1 = relu(W1 xT + b1), h2 = relu(W2 h1 + b2) over packed
  rows only (~52% of dense), activations transposed [feat, rows].
- Mean-pool: acc[f, s] = sum over planes of h2 columns (DVE adds over the
  plane staircase); pad columns contribute c2 = MLP2(0), computed on device
  from a zero column, cancelled exactly: pooled = (acc - (B_s - n_s) c2)/n_s.
- h3 = relu(W3 pooled + b3); y = relu(W4 h3 + b4) with the last matmul flipped
  so output lands sample-major; host undoes the sort/deal permutation.
- All matmuls in float32r (full PE rate, ~1e-4 rel err).
"""

import os
import sys
import types

import numpy as np
import concourse.mybir as mybir
import concourse.tile as tile
from concourse import bacc
from concourse.bass_utils import run_bass_kernel_spmd


def _ensure_ntff_hook():
    """If tracing is requested but the image lacks antenv.axon_hooks,
    install the ctypes-based NTFF hook so run_bass_kernel_spmd doesn't crash."""
    if not os.environ.get("BASS_TRACE"):
        return
    try:
        import antenv.axon_hooks  # noqa: F401
        return
    except ImportError:
        pass
    try:
        from trn_agent_boot.trn_boot import _ntff_profile_via_ctypes
        hook = _ntff_profile_via_ctypes("/opt/axon/libaxon_pjrt.so")
    except Exception:
        hook = None
    mod = types.ModuleType("antenv.axon_hooks")
    mod.get_axon_ntff_profile_hook = lambda: hook
    mod.set_axon_ntff_profile_hook = lambda h: None
    import antenv
    antenv.axon_hooks = mod
    sys.modules["antenv.axon_hooks"] = mod

S, M, D, H = 4096, 32, 420, 840
N_CORES = 8
SC = S // N_CORES            # samples per core (512)
TILE_N = 512                 # rows per tile
DP = 512                     # padded feature dim (d, e, f): 4 chunks of 128
HP = 896                     # padded hidden dim (h): 7 chunks of 128
NKD = DP // 128              # 4
NKH = HP // 128              # 7

F32 = mybir.dt.float32
F32R = mybir.dt.float32r

LAST_EXEC_NS = None
_PROGRAM_CACHE = {}


def _build_program(cnt):
    """cnt: tuple of plane widths (len M, non-increasing). Shared across cores."""
    cnt = [c for c in cnt if c > 0]
    offs = np.concatenate([[0], np.cumsum(cnt)])  # plane start offsets
    rows = int(offs[-1])
    r_pad = -(-rows // TILE_N) * TILE_N
    n_tiles = r_pad // TILE_N

    # tile t -> list of (fc-independent) adds: (j, col0, col1, tilecol0)
    tile_adds = [[] for _ in range(n_tiles)]
    for j, w in enumerate(cnt):
        o = int(offs[j])
        t0, t1 = o // TILE_N, (o + w - 1) // TILE_N
        for t in range(t0, t1 + 1):
            s0 = max(0, t * TILE_N - o)
            s1 = min(w, (t + 1) * TILE_N - o)
            tile_adds[t].append((s0, s1, o + s0 - t * TILE_N))

    nc = bacc.Bacc("TRN2", target_bir_lowering=False, debug=False)

    xT = nc.dram_tensor("xT", [DP, r_pad], F32R, kind="ExternalInput").ap()
    w1t = nc.dram_tensor("w1t", [DP, DP], F32R, kind="ExternalInput").ap()
    w2t = nc.dram_tensor("w2t", [DP, DP], F32R, kind="ExternalInput").ap()
    w3t = nc.dram_tensor("w3t", [DP, HP], F32R, kind="ExternalInput").ap()
    w4t = nc.dram_tensor("w4t", [HP, DP], F32R, kind="ExternalInput").ap()
    smw = 2 * NKD + NKH + D + 2 * SC
    smalls = nc.dram_tensor("smalls", [128, smw], F32, kind="ExternalInput").ap()
    zc = nc.dram_tensor("zc", [128, NKD * 256], F32R, kind="ExternalInput").ap()
    y = nc.dram_tensor("y", [SC, D], F32, kind="ExternalOutput").ap()

    relu = mybir.ActivationFunctionType.Relu

    with tile.TileContext(nc) as tc:
        with (
            tc.tile_pool(name="const", bufs=1) as const,
            tc.tile_pool(name="acc", bufs=1) as accp,
            tc.tile_pool(name="xin", bufs=4) as xin,
            tc.tile_pool(name="work", bufs=4) as work,
            tc.tile_pool(name="h2p", bufs=5) as h2p,
            tc.tile_pool(name="psum", bufs=8, space="PSUM") as psum,
        ):
            # ---- constants to SBUF via the Scalar HWDGE queue so the Sync
            # queue is free for x tiles (DMA issues serialize per queue) ----
            w1t_sb = const.tile([128, NKD, DP], F32R)
            w2t_sb = const.tile([128, NKD, DP], F32R)
            w3t_sb = const.tile([128, NKD, HP], F32R)
            w4t_sb = const.tile([128, NKH, DP], F32R)
            nc.scalar.dma_start(w1t_sb[:], w1t.rearrange("(c p) e -> p c e", p=128))
            sm_sb = const.tile([128, smw], F32)
            nc.scalar.dma_start(sm_sb[:], smalls)
            nc.scalar.dma_start(w2t_sb[:], w2t.rearrange("(c p) e -> p c e", p=128))
            nc.scalar.dma_start(w3t_sb[:], w3t.rearrange("(c p) e -> p c e", p=128))
            nc.scalar.dma_start(w4t_sb[:], w4t.rearrange("(c p) e -> p c e", p=128))
            o = 0
            b1_sb = sm_sb[:, o:o + NKD]; o += NKD
            b2_sb = sm_sb[:, o:o + NKD]; o += NKD
            b3_sb = sm_sb[:, o:o + NKH]; o += NKH
            b4_sb = sm_sb[:, o:o + D]; o += D
            inv_sb = sm_sb[:, o:o + SC]; o += SC
            beta_sb = sm_sb[:, o:o + SC]; o += SC

            # ---- PE warm-up: dummy bf16 matmuls bridge the DMA head so the
            # HAM clock-gate is at 8/8 when real work arrives ----
            wz = const.tile([128, 512], mybir.dt.bfloat16)
            nc.vector.memset(wz[:], 0.0)
            for wi in range(40):
                wps = psum.tile([128, 512], F32, tag="ps")
                nc.tensor.matmul(wps[:], wz[:, :128], wz[:],
                                 start=True, stop=True)

            # ---- pooled-sum accumulator ----
            acc = accp.tile([128, NKD, SC], F32)
            nc.any.memset(acc[:], 0.0)

            # ---- main loop over packed row tiles ----
            def l2_block(t, h1_t, tn):
                for fc in range(NKD):
                    ps = psum.tile([128, 512], F32, tag="ps")
                    for ec in range(NKD):
                        nc.tensor.matmul(
                            ps[:, :tn],
                            w2t_sb[:, ec, fc * 128:(fc + 1) * 128],
                            h1_t[:, ec, :tn],
                            start=(ec == 0), stop=(ec == NKD - 1),
                        )
                    h2_t = h2p.tile([128, TILE_N], F32, tag="h2")
                    nc.scalar.activation(h2_t[:, :tn], ps[:, :tn], relu,
                                         bias=b2_sb[:, fc:fc + 1])
                    for (s0, s1, tc0) in tile_adds[t]:
                        w = s1 - s0
                        nc.vector.tensor_add(
                            acc[:, fc, s0:s1],
                            acc[:, fc, s0:s1],
                            h2_t[:, tc0:tc0 + w],
                        )

            prev = None
            t1s = []
            c2_t = min(6, n_tiles - 1)
            t1_t = min(7, n_tiles - 1)
            for t in range(n_tiles):
                n0 = t * TILE_N
                tn = min(TILE_N, max(rows - n0, 256))
                x_t = xin.tile([128, NKD, TILE_N], F32R, tag="x")
                nc.sync.dma_start(
                    x_t[:, :, :tn],
                    xT[:, n0:n0 + tn].rearrange("(c p) n -> p c n", p=128),
                )
                h1_t = work.tile([128, NKD, TILE_N], F32R, tag="h1")
                for ec in range(NKD):
                    ps = psum.tile([128, 512], F32, tag="ps")
                    for kc in range(NKD):
                        nc.tensor.matmul(
                            ps[:, :tn],
                            w1t_sb[:, kc, ec * 128:(ec + 1) * 128],
                            x_t[:, kc, :tn],
                            start=(kc == 0), stop=(kc == NKD - 1),
                        )
                    nc.scalar.activation(h1_t[:, ec, :tn], ps[:, :tn], relu,
                                         bias=b1_sb[:, ec:ec + 1])
                if t == c2_t:
                    # ---- c2 = layer2(layer1(0)) on device (exact pad cancellation) ----
                    zcol = const.tile([128, NKD, 256], F32R)
                    nc.scalar.dma_start(zcol[:], zc.rearrange("p (c n) -> p c n", c=NKD))
                    h1c = const.tile([128, NKD, 256], F32R)
                    c2_sb = const.tile([128, NKD], F32)
                    for ec in range(NKD):
                        ps = psum.tile([128, 512], F32, tag="ps")
                        for kc in range(NKD):
                            nc.tensor.matmul(
                                ps[:, :256],
                                w1t_sb[:, kc, ec * 128:(ec + 1) * 128],
                                zcol[:, kc, :],
                                start=(kc == 0), stop=(kc == NKD - 1),
                            )
                        nc.scalar.activation(h1c[:, ec, :], ps[:, :256], relu,
                                             bias=b1_sb[:, ec:ec + 1])
                    for fc in range(NKD):
                        ps = psum.tile([128, 512], F32, tag="ps")
                        for ec in range(NKD):
                            nc.tensor.matmul(
                                ps[:, :256],
                                w2t_sb[:, ec, fc * 128:(fc + 1) * 128],
                                h1c[:, ec, :],
                                start=(ec == 0), stop=(ec == NKD - 1),
                            )
                        nc.scalar.activation(c2_sb[:, fc:fc + 1], ps[:, :1], relu,
                                             bias=b2_sb[:, fc:fc + 1])

                if t == t1_t:
                    for fc in range(NKD):
                        t1s.append(accp.tile([128, SC], F32, name=f"t1_{fc}"))
                        nc.vector.tensor_mul(
                            t1s[fc][:], beta_sb[:],
                            c2_sb[:, fc:fc + 1].to_broadcast((128, SC)),
                        )
                if prev is not None:
                    l2_block(*prev)

                prev = (t, h1_t, tn)

            l2_block(*prev)

            # ---- ragged correction: pooled = acc * inv_n - beta * c2 ----
            pooledT = accp.tile([128, NKD, SC], F32R)
            for fc in range(NKD):
                t2 = work.tile([128, SC], F32, tag="t2")
                nc.vector.tensor_mul(t2[:], acc[:, fc, :], inv_sb[:])
                nc.vector.tensor_sub(pooledT[:, fc, :], t2[:], t1s[fc][:])

            # ---- layer 3: h3T = relu(W3 pooled + b3), [HP, SC] ----
            h3T = accp.tile([128, NKH, SC], F32R)
            for hc in range(NKH):
                ps = psum.tile([128, 512], F32, tag="ps")
                for kc in range(NKD):
                    nc.tensor.matmul(
                        ps[:],
                        w3t_sb[:, kc, hc * 128:(hc + 1) * 128],
                        pooledT[:, kc, :],
                        start=(kc == 0), stop=(kc == NKD - 1),
                    )
                nc.scalar.activation(h3T[:, hc, :], ps[:], relu,
                                     bias=b3_sb[:, hc:hc + 1])

            # ---- layer 4 (flipped): y[s, :] = relu(h3T.T @ W4T + b4) ----
            for sc4 in range(SC // 128):
                ps = psum.tile([128, 512], F32, tag="ps")
                for hc in range(NKH):
                    nc.tensor.matmul(
                        ps[:, :D],
                        h3T[:, hc, sc4 * 128:(sc4 + 1) * 128],
                        w4t_sb[:, hc, :D],
                        start=(hc == 0), stop=(hc == NKH - 1),
                    )
                y_t = work.tile([128, D], F32, tag="yt")
                nc.vector.tensor_add(y_t[:], ps[:, :D], b4_sb[:])
                nc.vector.tensor_scalar_max(y_t[:], y_t[:], 0.0)
                nc.sync.dma_start(y[sc4 * 128:(sc4 + 1) * 128, :], y_t[:])

    nc.compile()
    return nc, rows, r_pad


def kernel(**inputs):
    global LAST_EXEC_NS
    all_x = np.asarray(inputs["all_x"], dtype=np.float32)
    numInputs = np.asarray(inputs["numInputs"]).astype(np.int64)
    W1 = np.asarray(inputs["W1"], dtype=np.float32)
    b1 = np.asarray(inputs["b1"], dtype=np.float32)
    W2 = np.asarray(inputs["W2"], dtype=np.float32)
    b2 = np.asarray(inputs["b2"], dtype=np.float32)
    W3 = np.asarray(inputs["W3"], dtype=np.float32)
    b3 = np.asarray(inputs["b3"], dtype=np.float32)
    W4 = np.asarray(inputs["W4"], dtype=np.float32)
    b4 = np.asarray(inputs["b4"], dtype=np.float32)

    # ---- shared ragged geometry: sort by n desc, deal round-robin ----
    perm = np.argsort(-numInputs, kind="stable")      # global rank -> sample
    G = np.array([(numInputs > j).sum() for j in range(M)])  # plane sizes
    cnt = np.ceil(G / N_CORES).astype(np.int64)       # shared per-core widths
    cnt_key = tuple(int(c) for c in cnt)
    B = np.array([(cnt > s).sum() for s in range(SC)], np.int64)  # slots/column

    cached = _PROGRAM_CACHE.get(cnt_key)
    if cached is None:
        cached = _build_program(cnt_key)
        _PROGRAM_CACHE[cnt_key] = cached
    nc, rows, r_pad = cached

    def padded_T(w, rows_to, cols_to=None):
        out_dim, in_dim = w.shape
        cols_to = cols_to or out_dim
        a = np.zeros((rows_to, cols_to), np.float32)
        a[:in_dim, :out_dim] = w.T
        return a

    w1t = padded_T(W1, DP, DP)
    w2t = padded_T(W2, DP, DP)
    w3t = padded_T(W3, DP, HP)
    w4t = padded_T(W4, HP, DP)

    def chunked_bias(b, pad_to, nchunks):
        a = np.zeros(pad_to, np.float32)
        a[:b.shape[0]] = b
        return np.ascontiguousarray(a.reshape(nchunks, 128).T)

    b1c = chunked_bias(b1, DP, NKD)
    b2c = chunked_bias(b2, DP, NKD)
    b3c = chunked_bias(b3, HP, NKH)
    b4rep = np.ascontiguousarray(np.broadcast_to(b4[None, :], (128, D)))

    x_flat = all_x.reshape(S * M, D)
    x_flat = np.vstack([x_flat, np.zeros((1, D), np.float32)])  # row -1 = zeros
    cnz = [j for j in range(M) if cnt[j] > 0]

    in_maps = []
    for c in range(N_CORES):
        # build packed row index list for this core
        idx_parts = []
        for j in cnz:
            w = int(cnt[j])
            scols = np.arange(w)
            ranks = N_CORES * scols + c
            valid = ranks < G[j]
            samp = np.where(valid, perm[np.minimum(ranks, S - 1)], -1)
            fi = np.where(valid, samp * M + j, S * M)  # S*M -> zero row
            idx_parts.append(fi)
        idx = np.concatenate(idx_parts)
        xp = x_flat[idx]                               # [rows, D]
        xT = np.zeros((DP, r_pad), np.float32)
        xT[:D, :rows] = xp.T

        ranks_c = N_CORES * np.arange(SC) + c
        n_col = numInputs[perm[ranks_c]].astype(np.float32)
        inv = (1.0 / n_col).astype(np.float32)
        beta = ((B - n_col) / n_col).astype(np.float32)
        smalls = np.concatenate([
            b1c, b2c, b3c, b4rep,
            np.broadcast_to(inv[None, :], (128, SC)),
            np.broadcast_to(beta[None, :], (128, SC)),
        ], axis=1).astype(np.float32)
        in_maps.append({
            "xT": xT,
            "w1t": w1t, "w2t": w2t, "w3t": w3t, "w4t": w4t,
            "zc": np.zeros((128, NKD * 256), np.float32),
            "smalls": np.ascontiguousarray(smalls),
        })

    _ensure_ntff_hook()
    res = run_bass_kernel_spmd(nc, in_maps, core_ids=list(range(N_CORES)))
    LAST_EXEC_NS = res.exec_time_ns

    out = np.empty((S, D), np.float32)
    for c in range(N_CORES):
        ranks_c = N_CORES * np.arange(SC) + c
        out[perm[ranks_c]] = res.results[c]["y"]
    return out[:, None, :].astype(np.float32)



# revision 23
# speedup vs baseline: 1.8258x; 1.0195x over previous
"""Trainium2 Bass kernel for nn_Dereverber_v3 (ragged MLP + masked mean-pool + MLP).

Strategy (pure data parallelism over 8 NeuronCores, ragged-packed):
- Sort samples by numInputs descending, deal round-robin to the 8 cores so all
  cores share one compile-time row geometry (per-core padding <= 32 rows).
- Pack valid (sample, slot) rows slot-major into "planes": plane j holds
  column s for every sample rank s with numInputs > j; plane widths cnt_j are
  a shared prefix staircase. Invalid/pad rows are zeroed.
- Device per core: h1 = relu(W1 xT + b1), h2 = relu(W2 h1 + b2) over packed
  rows only (~52% of dense), activations transposed [feat, rows].
- Mean-pool: acc[f, s] = sum over planes of h2 columns (DVE adds over the
  plane staircase); pad columns contribute c2 = MLP2(0), computed on device
  from a zero column, cancelled exactly: pooled = (acc - (B_s - n_s) c2)/n_s.
- h3 = relu(W3 pooled + b3); y = relu(W4 h3 + b4) with the last matmul flipped
  so output lands sample-major; host undoes the sort/deal permutation.
- All matmuls in float32r (full PE rate, ~1e-4 rel err).
"""

import os
import sys
import types

import numpy as np
import concourse.mybir as mybir
import concourse.tile as tile
from concourse import bacc
from concourse.bass_utils import run_bass_kernel_spmd


def _ensure_ntff_hook():
    """If tracing is requested but the image lacks antenv.axon_hooks,
    install the ctypes-based NTFF hook so run_bass_kernel_spmd doesn't crash."""
    if not os.environ.get("BASS_TRACE"):
        return
    try:
        import antenv.axon_hooks  # noqa: F401
        return
    except ImportError:
        pass
    try:
        from trn_agent_boot.trn_boot import _ntff_profile_via_ctypes
        hook = _ntff_profile_via_ctypes("/opt/axon/libaxon_pjrt.so")
    except Exception:
        hook = None
    mod = types.ModuleType("antenv.axon_hooks")
    mod.get_axon_ntff_profile_hook = lambda: hook
    mod.set_axon_ntff_profile_hook = lambda h: None
    import antenv
    antenv.axon_hooks = mod
    sys.modules["antenv.axon_hooks"] = mod

S, M, D, H = 4096, 32, 420, 840
N_CORES = 8
SC = S // N_CORES            # samples per core (512)
TILE_N = 512                 # rows per tile
DP = 512                     # padded feature dim (d, e, f): 4 chunks of 128
HP = 896                     # padded hidden dim (h): 7 chunks of 128
NKD = DP // 128              # 4
NKH = HP // 128              # 7

F32 = mybir.dt.float32
F32R = mybir.dt.float32r

LAST_EXEC_NS = None
_PROGRAM_CACHE = {}


def _build_program(cnt):
    """cnt: tuple of plane widths (len M, non-increasing). Shared across cores."""
    cnt = [c for c in cnt if c > 0]
    offs = np.concatenate([[0], np.cumsum(cnt)])  # plane start offsets
    rows = int(offs[-1])
    r_pad = -(-rows // TILE_N) * TILE_N
    n_tiles = r_pad // TILE_N

    # tile t -> list of (fc-independent) adds: (j, col0, col1, tilecol0)
    tile_adds = [[] for _ in range(n_tiles)]
    for j, w in enumerate(cnt):
        o = int(offs[j])
        t0, t1 = o // TILE_N, (o + w - 1) // TILE_N
        for t in range(t0, t1 + 1):
            s0 = max(0, t * TILE_N - o)
            s1 = min(w, (t + 1) * TILE_N - o)
            tile_adds[t].append((s0, s1, o + s0 - t * TILE_N))

    nc = bacc.Bacc("TRN2", target_bir_lowering=False, debug=False)

    xT = nc.dram_tensor("xT", [DP, r_pad], F32R, kind="ExternalInput").ap()
    w1t = nc.dram_tensor("w1t", [DP, DP], F32R, kind="ExternalInput").ap()
    w2t = nc.dram_tensor("w2t", [DP, DP], F32R, kind="ExternalInput").ap()
    w3t = nc.dram_tensor("w3t", [DP, HP], F32R, kind="ExternalInput").ap()
    w4t = nc.dram_tensor("w4t", [HP, DP], F32R, kind="ExternalInput").ap()
    smw = 2 * NKD + NKH + D + 2 * SC
    smalls = nc.dram_tensor("smalls", [128, smw], F32, kind="ExternalInput").ap()
    zc = nc.dram_tensor("zc", [128, NKD * 256], F32R, kind="ExternalInput").ap()
    y = nc.dram_tensor("y", [SC, D], F32, kind="ExternalOutput").ap()

    relu = mybir.ActivationFunctionType.Relu

    with tile.TileContext(nc) as tc:
        with (
            tc.tile_pool(name="const", bufs=1) as const,
            tc.tile_pool(name="acc", bufs=1) as accp,
            tc.tile_pool(name="xin", bufs=4) as xin,
            tc.tile_pool(name="work", bufs=4) as work,
            tc.tile_pool(name="h2p", bufs=5) as h2p,
            tc.tile_pool(name="psum", bufs=8, space="PSUM") as psum,
        ):
            # ---- constants to SBUF via the Scalar HWDGE queue so the Sync
            # queue is free for x tiles (DMA issues serialize per queue) ----
            w1t_sb = const.tile([128, NKD, DP], F32R)
            w2t_sb = const.tile([128, NKD, DP], F32R)
            w3t_sb = const.tile([128, NKD, HP], F32R)
            w4t_sb = const.tile([128, NKH, DP], F32R)
            nc.scalar.dma_start(w1t_sb[:], w1t.rearrange("(c p) e -> p c e", p=128))
            sm_sb = const.tile([128, smw], F32)
            nc.scalar.dma_start(sm_sb[:], smalls)
            nc.scalar.dma_start(w2t_sb[:], w2t.rearrange("(c p) e -> p c e", p=128))
            nc.scalar.dma_start(w3t_sb[:], w3t.rearrange("(c p) e -> p c e", p=128))
            nc.scalar.dma_start(w4t_sb[:], w4t.rearrange("(c p) e -> p c e", p=128))
            o = 0
            b1_sb = sm_sb[:, o:o + NKD]; o += NKD
            b2_sb = sm_sb[:, o:o + NKD]; o += NKD
            b3_sb = sm_sb[:, o:o + NKH]; o += NKH
            b4_sb = sm_sb[:, o:o + D]; o += D
            inv_sb = sm_sb[:, o:o + SC]; o += SC
            beta_sb = sm_sb[:, o:o + SC]; o += SC

            # ---- PE warm-up: dummy bf16 matmuls bridge the DMA head so the
            # HAM clock-gate is at 8/8 when real work arrives ----
            wz = const.tile([128, 512], mybir.dt.bfloat16)
            nc.vector.memset(wz[:], 0.0)
            for wi in range(40):
                wps = psum.tile([128, 512], F32, tag="ps")
                nc.tensor.matmul(wps[:], wz[:, :128], wz[:],
                                 start=True, stop=True)

            # ---- pooled-sum accumulator ----
            acc = accp.tile([128, NKD, SC], F32)
            nc.any.memset(acc[:], 0.0)

            # ---- main loop over packed row tiles ----
            def l2_block(t, h1_t, tn):
                for fc in range(NKD):
                    ps = psum.tile([128, 512], F32, tag="ps")
                    for ec in range(NKD):
                        nc.tensor.matmul(
                            ps[:, :tn],
                            w2t_sb[:, ec, fc * 128:(fc + 1) * 128],
                            h1_t[:, ec, :tn],
                            start=(ec == 0), stop=(ec == NKD - 1),
                        )
                    h2_t = h2p.tile([128, TILE_N], F32, tag="h2")
                    nc.scalar.activation(h2_t[:, :tn], ps[:, :tn], relu,
                                         bias=b2_sb[:, fc:fc + 1])
                    for (s0, s1, tc0) in tile_adds[t]:
                        w = s1 - s0
                        nc.vector.tensor_add(
                            acc[:, fc, s0:s1],
                            acc[:, fc, s0:s1],
                            h2_t[:, tc0:tc0 + w],
                        )

            prev = None
            t1s = []
            c2_t = min(6, n_tiles - 1)
            t1_t = min(7, n_tiles - 1)
            for t in range(n_tiles):
                n0 = t * TILE_N
                tn = min(TILE_N, max(rows - n0, 256))
                x_t = xin.tile([128, NKD, TILE_N], F32R, tag="x")
                nc.sync.dma_start(
                    x_t[:, :, :tn],
                    xT[:, n0:n0 + tn].rearrange("(c p) n -> p c n", p=128),
                )
                h1_t = work.tile([128, NKD, TILE_N], F32R, tag="h1")
                for ec in range(NKD):
                    ps = psum.tile([128, 512], F32, tag="ps")
                    for kc in range(NKD):
                        nc.tensor.matmul(
                            ps[:, :tn],
                            w1t_sb[:, kc, ec * 128:(ec + 1) * 128],
                            x_t[:, kc, :tn],
                            start=(kc == 0), stop=(kc == NKD - 1),
                        )
                    nc.scalar.activation(h1_t[:, ec, :tn], ps[:, :tn], relu,
                                         bias=b1_sb[:, ec:ec + 1])
                if t == c2_t:
                    # ---- c2 = layer2(layer1(0)) on device (exact pad cancellation) ----
                    zcol = const.tile([128, NKD, 256], F32R)
                    nc.scalar.dma_start(zcol[:], zc.rearrange("p (c n) -> p c n", c=NKD))
                    h1c = const.tile([128, NKD, 256], F32R)
                    c2_sb = const.tile([128, NKD], F32)
                    for ec in range(NKD):
                        ps = psum.tile([128, 512], F32, tag="ps")
                        for kc in range(NKD):
                            nc.tensor.matmul(
                                ps[:, :256],
                                w1t_sb[:, kc, ec * 128:(ec + 1) * 128],
                                zcol[:, kc, :],
                                start=(kc == 0), stop=(kc == NKD - 1),
                            )
                        nc.scalar.activation(h1c[:, ec, :], ps[:, :256], relu,
                                             bias=b1_sb[:, ec:ec + 1])
                    for fc in range(NKD):
                        ps = psum.tile([128, 512], F32, tag="ps")
                        for ec in range(NKD):
                            nc.tensor.matmul(
                                ps[:, :256],
                                w2t_sb[:, ec, fc * 128:(fc + 1) * 128],
                                h1c[:, ec, :],
                                start=(ec == 0), stop=(ec == NKD - 1),
                            )
                        nc.scalar.activation(c2_sb[:, fc:fc + 1], ps[:, :1], relu,
                                             bias=b2_sb[:, fc:fc + 1])

                if t == t1_t:
                    for fc in range(NKD):
                        t1s.append(accp.tile([128, SC], F32, name=f"t1_{fc}"))
                        nc.vector.tensor_mul(
                            t1s[fc][:], beta_sb[:],
                            c2_sb[:, fc:fc + 1].to_broadcast((128, SC)),
                        )
                if prev is not None:
                    l2_block(*prev)

                prev = (t, h1_t, tn)

            l2_block(*prev)

            # keep PE busy (HAM warm) while the DVE correction chain runs
            for wi in range(16):
                wps = psum.tile([128, 512], F32, tag="ps", name=f"warm2_{wi}")
                nc.tensor.matmul(wps[:], wz[:, :128], wz[:],
                                 start=True, stop=True)

            # ---- ragged correction: pooled = acc * inv_n - beta * c2 ----
            pooledT = accp.tile([128, NKD, SC], F32R)
            for fc in range(NKD):
                t2 = work.tile([128, SC], F32, tag="t2")
                nc.vector.tensor_mul(t2[:], acc[:, fc, :], inv_sb[:])
                nc.vector.tensor_sub(pooledT[:, fc, :], t2[:], t1s[fc][:])

            # ---- layer 3 (kc-major): starts as soon as pooled[0] is ready ----
            h3T = accp.tile([128, NKH, SC], F32R)
            l3ps = []
            for hc in range(NKH):
                l3ps.append(psum.tile([128, 512], F32, tag="ps", name=f"l3ps_{hc}"))
            for kc in range(NKD):
                for hc in range(NKH):
                    nc.tensor.matmul(
                        l3ps[hc][:],
                        w3t_sb[:, kc, hc * 128:(hc + 1) * 128],
                        pooledT[:, kc, :],
                        start=(kc == 0), stop=(kc == NKD - 1),
                    )
            for hc in range(NKH):
                nc.scalar.activation(h3T[:, hc, :], l3ps[hc][:], relu,
                                     bias=b3_sb[:, hc:hc + 1])

            # ---- layer 4 (flipped): y[s, :] = relu(h3T.T @ W4T + b4) ----
            for sc4 in range(SC // 128):
                ps = psum.tile([128, 512], F32, tag="ps")
                for hc in range(NKH):
                    nc.tensor.matmul(
                        ps[:, :D],
                        h3T[:, hc, sc4 * 128:(sc4 + 1) * 128],
                        w4t_sb[:, hc, :D],
                        start=(hc == 0), stop=(hc == NKH - 1),
                    )
                y_t = work.tile([128, D], F32, tag="yt")
                nc.vector.tensor_add(y_t[:], ps[:, :D], b4_sb[:])
                nc.vector.tensor_scalar_max(y_t[:], y_t[:], 0.0)
                nc.sync.dma_start(y[sc4 * 128:(sc4 + 1) * 128, :], y_t[:])

    nc.compile()
    return nc, rows, r_pad


def kernel(**inputs):
    global LAST_EXEC_NS
    all_x = np.asarray(inputs["all_x"], dtype=np.float32)
    numInputs = np.asarray(inputs["numInputs"]).astype(np.int64)
    W1 = np.asarray(inputs["W1"], dtype=np.float32)
    b1 = np.asarray(inputs["b1"], dtype=np.float32)
    W2 = np.asarray(inputs["W2"], dtype=np.float32)
    b2 = np.asarray(inputs["b2"], dtype=np.float32)
    W3 = np.asarray(inputs["W3"], dtype=np.float32)
    b3 = np.asarray(inputs["b3"], dtype=np.float32)
    W4 = np.asarray(inputs["W4"], dtype=np.float32)
    b4 = np.asarray(inputs["b4"], dtype=np.float32)

    # ---- shared ragged geometry: sort by n desc, deal round-robin ----
    perm = np.argsort(-numInputs, kind="stable")      # global rank -> sample
    G = np.array([(numInputs > j).sum() for j in range(M)])  # plane sizes
    cnt = np.ceil(G / N_CORES).astype(np.int64)       # shared per-core widths
    cnt_key = tuple(int(c) for c in cnt)
    B = np.array([(cnt > s).sum() for s in range(SC)], np.int64)  # slots/column

    cached = _PROGRAM_CACHE.get(cnt_key)
    if cached is None:
        cached = _build_program(cnt_key)
        _PROGRAM_CACHE[cnt_key] = cached
    nc, rows, r_pad = cached

    def padded_T(w, rows_to, cols_to=None):
        out_dim, in_dim = w.shape
        cols_to = cols_to or out_dim
        a = np.zeros((rows_to, cols_to), np.float32)
        a[:in_dim, :out_dim] = w.T
        return a

    w1t = padded_T(W1, DP, DP)
    w2t = padded_T(W2, DP, DP)
    w3t = padded_T(W3, DP, HP)
    w4t = padded_T(W4, HP, DP)

    def chunked_bias(b, pad_to, nchunks):
        a = np.zeros(pad_to, np.float32)
        a[:b.shape[0]] = b
        return np.ascontiguousarray(a.reshape(nchunks, 128).T)

    b1c = chunked_bias(b1, DP, NKD)
    b2c = chunked_bias(b2, DP, NKD)
    b3c = chunked_bias(b3, HP, NKH)
    b4rep = np.ascontiguousarray(np.broadcast_to(b4[None, :], (128, D)))

    x_flat = all_x.reshape(S * M, D)
    x_flat = np.vstack([x_flat, np.zeros((1, D), np.float32)])  # row -1 = zeros
    cnz = [j for j in range(M) if cnt[j] > 0]

    in_maps = []
    for c in range(N_CORES):
        # build packed row index list for this core
        idx_parts = []
        for j in cnz:
            w = int(cnt[j])
            scols = np.arange(w)
            ranks = N_CORES * scols + c
            valid = ranks < G[j]
            samp = np.where(valid, perm[np.minimum(ranks, S - 1)], -1)
            fi = np.where(valid, samp * M + j, S * M)  # S*M -> zero row
            idx_parts.append(fi)
        idx = np.concatenate(idx_parts)
        xp = x_flat[idx]                               # [rows, D]
        xT = np.zeros((DP, r_pad), np.float32)
        xT[:D, :rows] = xp.T

        ranks_c = N_CORES * np.arange(SC) + c
        n_col = numInputs[perm[ranks_c]].astype(np.float32)
        inv = (1.0 / n_col).astype(np.float32)
        beta = ((B - n_col) / n_col).astype(np.float32)
        smalls = np.concatenate([
            b1c, b2c, b3c, b4rep,
            np.broadcast_to(inv[None, :], (128, SC)),
            np.broadcast_to(beta[None, :], (128, SC)),
        ], axis=1).astype(np.float32)
        in_maps.append({
            "xT": xT,
            "w1t": w1t, "w2t": w2t, "w3t": w3t, "w4t": w4t,
            "zc": np.zeros((128, NKD * 256), np.float32),
            "smalls": np.ascontiguousarray(smalls),
        })

    _ensure_ntff_hook()
    res = run_bass_kernel_spmd(nc, in_maps, core_ids=list(range(N_CORES)))
    LAST_EXEC_NS = res.exec_time_ns

    out = np.empty((S, D), np.float32)
    for c in range(N_CORES):
        ranks_c = N_CORES * np.arange(SC) + c
        out[perm[ranks_c]] = res.results[c]["y"]
    return out[:, None, :].astype(np.float32)

